# revision 12
# baseline (speedup 1.0000x reference)
"""Trainium2 Bass kernel for nn_InteractionGate (gnn_message_passing).

Contract: kernel(**inputs) takes the FULL unsharded inputs (as in
reference.setup_inputs()) and returns the FULL [1024, 1024, 64] output.
Internally shards the pairwise row dimension i across 8 NeuronCores
(128 rows each), runs one SPMD Bass/Tile program on cores 0-7, gathers.

Math: with
  W1 = w_gate[0:64], W2 = w_gate[64:128], W3 = w_gate[128:144], W4 = w_gate[144:160]
  u3 = w_dist @ W3, u4 = w_dist @ W4
  B  = AH @ (W1+W2) + b_dist @ (W3+W4) + b_gate          [N,H]
the reference reduces (off-diagonal) to
  out[i,j,h] = AH[j,h] * sigmoid(B[j,h] + diagv[i]*u3[h] + dist[i,j]*u4[h])
where dist is the cal_dist "distance_other" matrix. The diagonal entries
use GH instead of AH and are patched on the host (O(N*H) work).

Device plan per core (rows i in its 128-block, partitions = i):
  1. PE computes the five pairwise numerator matrices (each is rank<=6:
     sum_k f_k(i) g_k(j)) as K=6 fp32 matmuls (cancellation-sensitive).
  2. DVE/ACT compute dist[i,j] [128,1024] elementwise (reciprocal, sqrt,
     branch masks via predicated copies).
  3. PE transposes dist into dT33 [33, 4096] (row 32 = diagv row) via 8
     128x128 transposes; PSUM evictions round to float32r.
  4. Main loop over 64 half-chunks (16 j's x 64 h = 1024 free each),
     all matmuls in float32r (1 PE cycle/column, 11-bit mantissa):
     PE:  logit  = dT33-chunk.T @ [delta*u4 ; u3row]  (K=33)
                 + ones.T @ B_row-slice               (K=1 broadcast)
          ah     = ones.T @ AH_row-slice              (K=1 broadcast)
     ACT: sig = sigmoid(logit)   (PSUM -> SBUF)
     DVE: out = sig * ah         (SBUF x PSUM -> SBUF)
     DMA: out tile (2 windows batched = 1 MiB) -> HBM.
"""
import os
import sys
from contextlib import ExitStack

import numpy as np

if "/opt/trn_rl_repo" not in sys.path:
    sys.path.insert(0, "/opt/trn_rl_repo")

import concourse.bass as bass
import concourse.bacc as bacc
import concourse.mybir as mybir
import concourse.tile as tile
from concourse import bass_utils

N, H, E = 1024, 64, 16
NCORES = 8
R = N // NCORES            # 128 rows per core
F32 = mybir.dt.float32
F32R = mybir.dt.float32r
AF = mybir.ActivationFunctionType
OP = mybir.AluOpType

NJ_CHUNK = 32              # j's per K-matmul chunk (lhsT partition rows)
NCHUNK = N // NJ_CHUNK     # 32 chunks
NJ_HALF = 16               # j's per PSUM window
WFREE = NJ_HALF * H        # 1024 free elements per window
NHALF = N // NJ_HALF       # 64 windows per core
OUT_BATCH = 2              # windows per output DMA (1 MiB per DMA)


def _sigmoid(x):
    return 1.0 / (1.0 + np.exp(-x))


def _fp32r(x):
    """Round fp32 -> fp32r (11 mantissa bits, round-half-even) like the PE."""
    b = np.ascontiguousarray(x, np.float32).view(np.uint32)
    rb = (b + np.uint32(0x7FF) + ((b >> np.uint32(12)) & np.uint32(1))) \
        & np.uint32(0xFFFFF000)
    return rb.view(np.float32)


def _host_prep(action_hidden_state, goal_hidden_state, goal, action,
               w_dist, b_dist, w_gate, b_gate):
    f32 = np.float32
    AH = np.ascontiguousarray(action_hidden_state, f32)
    GH = np.ascontiguousarray(goal_hidden_state, f32)
    goal = np.asarray(goal, f32)
    action = np.asarray(action, f32)
    w_dist = np.asarray(w_dist, f32)
    b_dist = np.asarray(b_dist, f32)
    w_gate = np.asarray(w_gate, f32)
    b_gate = np.asarray(b_gate, f32)

    ax, ay = action[:, 0].copy(), action[:, 1].copy()
    gx, gy = goal[:, 0].copy(), goal[:, 1].copy()
    gyx = gy - gx
    diagv = np.sqrt((ax - gx) ** 2 + (ay - gy) ** 2).astype(f32)

    W1, W2 = w_gate[0:H], w_gate[H:2 * H]
    W3, W4 = w_gate[2 * H:2 * H + E], w_gate[2 * H + E:2 * H + 2 * E]
    u3 = (w_dist @ W3).astype(f32)
    u4 = (w_dist @ W4).astype(f32)
    B = (AH @ (W1 + W2) + b_dist @ (W3 + W4) + b_gate).astype(f32)

    one = np.ones(N, f32)
    # rank factors: num[i,j] = sum_k f[k][i] * g[k][j]
    f_cav = np.stack([ax, -ax * gx, -ay, ay * gx])
    g_cav = np.stack([ay * gx, ay, ax * gx, ax])
    f_caz = np.stack([ax, -ax * gy, -ay, ay * gy])
    g_caz = np.stack([ay * gy, ay, ax * gy, ax])
    f_wcg1 = np.stack([gx, -ax * gx]); g_wcg1 = np.stack([ax * gyx, gyx])
    f_wcg2 = np.stack([gyx, -ax * gyx]); g_wcg2 = np.stack([ax * gx, gx])
    f_scg1 = np.stack([gx, -ay * gx]); g_scg1 = np.stack([ax * gyx, gyx])
    f_t2 = np.stack([gyx, -ax * gyx]); g_t2 = np.stack([ay * gx, gx])
    f_dnm = np.stack([one, -ay, -gx, ay * gx, np.zeros(N, f32), np.zeros(N, f32)])
    g_dnm = np.stack([ay * gx, gx, ay, one, np.zeros(N, f32), np.zeros(N, f32)])

    fg = dict(
        dnm=(f_dnm, g_dnm),
        num1=(np.concatenate([f_cav, -f_wcg1]), np.concatenate([g_cav, g_wcg1])),
        num1p=(np.concatenate([f_cav, f_wcg2]), np.concatenate([g_cav, g_wcg2])),
        num2=(np.concatenate([f_caz, -f_scg1]), np.concatenate([g_caz, g_scg1])),
        num2p=(np.concatenate([f_caz, f_t2]), np.concatenate([g_caz, g_t2])),
    )

    logit_diag = (B + (GH - AH) @ W2 + diagv[:, None] * (u3 + u4)).astype(f32)
    out_diag = (GH * _sigmoid(logit_diag)).astype(f32)

    # rhs18 static rows: 0..15 delta(j_local)*u4 over a 16-j window, 16 = u3
    rhs18s = np.zeros((17, WFREE), f32)
    for jl in range(NJ_HALF):
        rhs18s[jl, jl * H:(jl + 1) * H] = u4
    rhs18s[16] = np.tile(u3, NJ_HALF)
    rhs18s = _fp32r(rhs18s)

    # per-window B/AH rows (row 17 of rhs18 / K=1 broadcast rhs)
    B_rows = _fp32r(B.reshape(NHALF, WFREE))
    AH_rows = _fp32r(AH.reshape(NHALF, WFREE))

    ones1 = np.ones((1, R), f32)

    return dict(AH=AH, GH=GH, ax=ax, ay=ay, gx=gx, gy=gy, diagv=diagv,
                u3=u3, u4=u4, B=B, fg=fg, out_diag=out_diag,
                B_rows=B_rows, AH_rows=AH_rows, rhs18s=rhs18s, ones1=ones1)


NUM_NAMES = ["dnm", "num1", "num1p", "num2", "num2p"]


def _core_inputs(prep, core):
    """Build the per-core in_map (numpy arrays for every ExternalInput)."""
    f32 = np.float32
    i0 = core * R
    sl = slice(i0, i0 + R)

    sc = np.zeros((R, 8), f32)
    sc[:, 0] = prep["ax"][sl]
    sc[:, 1] = prep["ay"][sl]
    sc[:, 2] = prep["gx"][sl]
    sc[:, 3] = prep["diagv"][sl]

    jj = np.arange(N)[None, :]
    ii = np.arange(i0, i0 + R)[:, None]
    mju = (jj > ii).astype(f32)
    mjl = (jj < ii).astype(f32)

    axj_b = np.broadcast_to(prep["ax"], (R, N)).copy()
    gxj_b = np.broadcast_to(prep["gx"], (R, N)).copy()

    ident = np.eye(128, dtype=f32)

    dvi_rep = _fp32r(np.tile(prep["diagv"][sl], NHALF)[None, :])  # [1, 64*128]
    ones_row = np.ones((1, NHALF * 128), f32)

    meye = (jj == ii).astype(f32)
    m = dict(sc=sc, mju=mju, mjl=mjl, meye=meye, axj_b=axj_b, gxj_b=gxj_b,
             ident=ident, dvi_rep=dvi_rep, ones_row=ones_row,
             rhs18s=prep["rhs18s"], ones1=prep["ones1"],
             B_rows=prep["B_rows"], AH_rows=prep["AH_rows"])
    for nm in NUM_NAMES:
        f, g = prep["fg"][nm]
        m[f"lhsT_{nm}"] = np.ascontiguousarray(f[:, sl].astype(f32))  # [6, 128]
        m[f"rhs_{nm}"] = np.ascontiguousarray(g.astype(f32))          # [6, 1024]
    return m


def _declare_tensors(nc):
    t = {}
    def inp(name, shape, dt=F32):
        t[name] = nc.dram_tensor(name, shape, dt, kind="ExternalInput").ap()
    inp("sc", [R, 8])
    inp("mju", [R, N]); inp("mjl", [R, N]); inp("meye", [R, N])
    inp("axj_b", [R, N]); inp("gxj_b", [R, N])
    inp("ident", [128, 128])
    inp("dvi_rep", [1, NHALF * 128], F32R)
    inp("ones_row", [1, NHALF * 128], F32R)
    inp("rhs18s", [17, WFREE], F32R)
    inp("ones1", [1, R], F32R)
    inp("B_rows", [NHALF, WFREE], F32R)
    inp("AH_rows", [NHALF, WFREE], F32R)
    for nm in NUM_NAMES:
        inp(f"lhsT_{nm}", [6, 128])
        inp(f"rhs_{nm}", [6, N])
    t["out"] = nc.dram_tensor("out", [R, N * H], F32, kind="ExternalOutput").ap()
    return t


def _build_program(ctx, tc, t):
    nc = tc.nc

    consts = ctx.enter_context(tc.tile_pool(name="consts", bufs=1))

    def load(name, shape, dt=F32):
        tl = consts.tile(shape, dt, tag=name, name=name)
        nc.sync.dma_start(tl[:], t[name])
        return tl

    sc = load("sc", [R, 8])
    mju = load("mju", [R, N])
    mjl = load("mjl", [R, N])
    meye = load("meye", [R, N])
    axj_b = load("axj_b", [R, N])
    gxj_b = load("gxj_b", [R, N])
    ident = load("ident", [128, 128])
    rhs18s = load("rhs18s", [17, WFREE], F32R)
    ones1 = load("ones1", [1, R], F32R)
    lhsT_num = {nm: load(f"lhsT_{nm}", [6, 128]) for nm in NUM_NAMES}
    rhs_num = {nm: load(f"rhs_{nm}", [6, N]) for nm in NUM_NAMES}

    AXi, AYi, GXi, DVi = (sc[:, k:k + 1] for k in range(4))

    # ---- phase 1: numerators via PE (rank<=6), eviction to SBUF ----
    nums = ctx.enter_context(tc.tile_pool(name="nums", bufs=1))
    work = ctx.enter_context(tc.tile_pool(name="work", bufs=1))
    num_sb = {}
    with tc.tile_pool(name="ps_num", bufs=2, space="PSUM") as ps_num:
        for nm in NUM_NAMES:
            ps = ps_num.tile([R, N], F32, tag="ps_num", name="ps_num")
            for w in range(N // 512):
                nc.tensor.matmul(ps[:, w * 512:(w + 1) * 512],
                                 lhsT_num[nm][:, :],
                                 rhs_num[nm][:, w * 512:(w + 1) * 512],
                                 start=True, stop=True)
            sb = nums.tile([R, N], F32, tag=f"num_{nm}", name=f"num_{nm}")
            nc.vector.tensor_copy(sb[:], ps[:])
            num_sb[nm] = sb

    # ---- phase 2: dist [128, 1024] elementwise ----
    def wtile():
        return work.tile([R, N], F32, tag="w", name="w", bufs=8)

    # p's overwrite their numerator tiles in place; rdn overwrites dnm
    rdn = num_sb["dnm"]
    nc.gpsimd.tensor_add(rdn[:], rdn[:], meye[:])
    nc.vector.reciprocal(rdn[:], rdn[:])
    p1, p2, p1p, p2p = (num_sb[k] for k in ("num1", "num2", "num1p", "num2p"))
    nc.vector.tensor_mul(p1[:], p1[:], rdn[:])
    nc.vector.tensor_mul(p2[:], p2[:], rdn[:])
    nc.vector.tensor_mul(p1p[:], p1p[:], rdn[:])
    nc.vector.tensor_mul(p2p[:], p2p[:], rdn[:])

    e1 = wtile()
    nc.vector.tensor_scalar(e1[:], p1[:], AXi, None, OP.subtract)
    q1 = wtile()
    nc.vector.scalar_tensor_tensor(q1[:], p1[:], GXi, e1[:], OP.subtract, OP.mult)
    e1s = wtile()
    nc.scalar.square(e1s[:], e1[:])
    e2 = e1  # e1 dead
    nc.vector.tensor_scalar(e2[:], p2[:], AYi, None, OP.subtract)
    e2s = p1  # p1 dead
    nc.scalar.square(e2s[:], e2[:])
    s12 = e2
    nc.vector.tensor_add(s12[:], e1s[:], e2s[:])
    d1p = wtile()
    nc.scalar.sqrt(d1p[:], s12[:])
    c1m = e1s
    nc.vector.tensor_scalar(c1m[:], q1[:], 0.0, None, OP.is_lt)
    m1 = q1
    nc.gpsimd.tensor_mul(m1[:], c1m[:], mju[:])

    g1 = s12
    nc.vector.tensor_scalar(g1[:], p1p[:], AXi, None, OP.subtract)
    g1s = c1m
    nc.scalar.square(g1s[:], g1[:])
    g2 = g1
    nc.vector.tensor_scalar(g2[:], p2p[:], AYi, None, OP.subtract)
    g2s = p2  # p2 dead
    nc.scalar.square(g2s[:], g2[:])
    s34 = g2
    nc.vector.tensor_add(s34[:], g1s[:], g2s[:])
    d2p = wtile()
    nc.scalar.sqrt(d2p[:], s34[:])

    t1 = g1s
    nc.gpsimd.tensor_sub(t1[:], p1p[:], axj_b[:])
    t2 = g2s
    nc.gpsimd.tensor_sub(t2[:], p1p[:], gxj_b[:])
    q2 = p1p  # p1p dead
    nc.gpsimd.tensor_mul(q2[:], t1[:], t2[:])
    c2m = t1
    nc.vector.tensor_scalar(c2m[:], q2[:], 0.0, None, OP.is_lt)
    m2 = t2
    nc.gpsimd.tensor_mul(m2[:], c2m[:], mjl[:])

    # walrus requires integer mask dtype for CopyPredicated
    mu1 = work.tile([R, N], mybir.dt.uint8, tag="mu1", name="mu1")
    mu2 = work.tile([R, N], mybir.dt.uint8, tag="mu2", name="mu2")
    nc.vector.tensor_copy(mu1[:], m1[:])
    nc.vector.tensor_copy(mu2[:], m2[:])

    dist = nums.tile([R, N], F32, tag="dist", name="dist")
    nc.vector.tensor_scalar(dist[:], mju[:], 0.0, DVi, OP.mult, OP.add)
    nc.vector.copy_predicated(dist[:], mu1[:], d1p[:])
    nc.vector.copy_predicated(dist[:], mu2[:], d2p[:])

    # ---- phase 3: transpose dist -> dT18 [18, NHALF*128] (float32r) ----
    # dT18[q, hh*128 + i] = dist[i, 16*hh + q]; row 16 = diagv rep, 17 = ones
    dT18 = nums.tile([18, NHALF * 128], F32R, tag="dT18", name="dT18")
    dT_sb = nums.tile([128, N], F32R, tag="dT_sb", name="dT_sb")
    with tc.tile_pool(name="ps_tr", bufs=2, space="PSUM") as ps_tr:
        for tt in range(8):
            ps = ps_tr.tile([128, 128], F32, tag="ps_tr", name="ps_tr")
            nc.tensor.transpose(ps[:], dist[:, tt * 128:(tt + 1) * 128], ident[:])
            nc.vector.tensor_copy(dT_sb[:, tt * 128:(tt + 1) * 128], ps[:])
    # relayout: dT18[q, (8g+b)*128 + i] = dT_sb[16b+q, 128g + i]
    dT18_v = dT18[0:16, :].rearrange("p (hh i) -> p hh i", i=128)
    dT_sb_v = dT_sb[:, :].rearrange("p (g i) -> p g i", i=128)
    for b in range(8):
        nc.sync.dma_start(dT18_v[:, b::8, :], dT_sb_v[16 * b:16 * (b + 1), :, :])
    nc.sync.dma_start(dT18[16:17, :], t["dvi_rep"])
    nc.sync.dma_start(dT18[17:18, :], t["ones_row"])

    # rhs18 double buffers: static rows 0..16 loaded once, row 17 per window
    rhs18_bufs = []
    for bi in range(2):
        rb = consts.tile([18, WFREE], F32R, tag=f"rhs18_{bi}", name=f"rhs18_{bi}")
        nc.sync.dma_start(rb[0:17, :], t["rhs18s"])
        rhs18_bufs.append(rb)

    # ---- phase 4: main loop over 64 16-j windows ----
    ps_logit = ctx.enter_context(tc.tile_pool(name="ps_logit", bufs=2, space="PSUM"))
    ps_ah = ctx.enter_context(tc.tile_pool(name="ps_ah", bufs=2, space="PSUM"))
    sig_pool = ctx.enter_context(tc.tile_pool(name="sig", bufs=3))
    out_pool = ctx.enter_context(tc.tile_pool(name="outsb", bufs=2))
    ahrow_pool = ctx.enter_context(tc.tile_pool(name="ahrow", bufs=2))

    out_sb = None
    for hh in range(NHALF):
        rb = rhs18_bufs[hh % 2]
        nc.sync.dma_start(rb[17:18, :], t["B_rows"][hh:hh + 1, :])
        ahrow = ahrow_pool.tile([1, WFREE], F32R, tag="ahrow", name="ahrow")
        nc.sync.dma_start(ahrow[:], t["AH_rows"][hh:hh + 1, :])

        lg = ps_logit.tile([R, WFREE], F32, tag="lg", name="lg")
        ah = ps_ah.tile([R, WFREE], F32, tag="ah", name="ah")
        for w in range(2):
            dst = slice(w * 512, (w + 1) * 512)
            nc.tensor.matmul(lg[:, dst], dT18[0:18, hh * 128:(hh + 1) * 128],
                             rb[:, dst], start=True, stop=True)
            nc.tensor.matmul(ah[:, dst], ones1[0:1, :], ahrow[0:1, dst],
                             start=True, stop=True)

        sig = sig_pool.tile([R, WFREE], F32, tag="sig", name="sig")
        nc.scalar.activation(sig[:], lg[:], AF.Sigmoid)

        if hh % OUT_BATCH == 0:
            out_sb = out_pool.tile([R, OUT_BATCH * WFREE], F32, tag="out_sb", name="out_sb")
        seg = slice((hh % OUT_BATCH) * WFREE, (hh % OUT_BATCH + 1) * WFREE)
        nc.vector.tensor_mul(out_sb[:, seg], sig[:], ah[:])
        if hh % OUT_BATCH == OUT_BATCH - 1:
            base = (hh - (OUT_BATCH - 1)) * WFREE
            nc.sync.dma_start(t["out"][:, base:base + OUT_BATCH * WFREE],
                              out_sb[:])


def build_nc():
    nc = bacc.Bacc("TRN2", target_bir_lowering=False, debug=False,
                   enable_asserts=False, num_devices=NCORES)
    t = _declare_tensors(nc)
    with tile.TileContext(nc) as tc:
        with ExitStack() as ctx:
            _build_program(ctx, tc, t)
    nc.compile()
    return nc


def kernel(**inputs):
    prep = _host_prep(**inputs)
    nc = build_nc()
    in_maps = [_core_inputs(prep, c) for c in range(NCORES)]
    res = bass_utils.run_bass_kernel_spmd(nc, in_maps, core_ids=list(range(NCORES)))
    out = np.concatenate([r["out"] for r in res.results], 0).reshape(N, N, H)
    # patch the diagonal (host-computed, uses GH and the diag logit)
    out[np.arange(N), np.arange(N)] = prep["out_diag"]
    return out


if __name__ == "__main__":
    import reference
    inputs = {k: np.asarray(v) for k, v in reference.setup_inputs().items()}
    out = kernel(**inputs)
    print("kernel out", out.shape, out.dtype)


# revision 21
# speedup vs baseline: 1.1701x; 1.1701x over previous
"""Trainium2 Bass kernel for nn_InteractionGate (gnn_message_passing).

Contract: kernel(**inputs) takes the FULL unsharded inputs (as in
reference.setup_inputs()) and returns the FULL [1024, 1024, 64] output.
Internally shards the pairwise row dimension i across 8 NeuronCores
(128 rows each), runs one SPMD Bass/Tile program on cores 0-7, gathers.

Math: with
  W1 = w_gate[0:64], W2 = w_gate[64:128], W3 = w_gate[128:144], W4 = w_gate[144:160]
  u3 = w_dist @ W3, u4 = w_dist @ W4
  B  = AH @ (W1+W2) + b_dist @ (W3+W4) + b_gate          [N,H]
the reference reduces (off-diagonal) to
  out[i,j,h] = AH[j,h] * sigmoid(B[j,h] + diagv[i]*u3[h] + dist[i,j]*u4[h])
where dist is the cal_dist "distance_other" matrix. The diagonal entries
use GH instead of AH and are patched on the host (O(N*H) work).

Device plan per core (rows i in its 128-block, partitions = i):
  1. PE computes the five pairwise numerator matrices (each is rank<=6:
     sum_k f_k(i) g_k(j)) as K=6 fp32 matmuls (cancellation-sensitive).
  2. DVE/ACT compute dist[i,j] [128,1024] elementwise (reciprocal, sqrt,
     branch masks via predicated copies).
  3. PE transposes dist into dT33 [33, 4096] (row 32 = diagv row) via 8
     128x128 transposes; PSUM evictions round to float32r.
  4. Main loop over 64 half-chunks (16 j's x 64 h = 1024 free each),
     all matmuls in float32r (1 PE cycle/column, 11-bit mantissa):
     PE:  logit  = dT33-chunk.T @ [delta*u4 ; u3row]  (K=33)
                 + ones.T @ B_row-slice               (K=1 broadcast)
          ah     = ones.T @ AH_row-slice              (K=1 broadcast)
     ACT: sig = sigmoid(logit)   (PSUM -> SBUF)
     DVE: out = sig * ah         (SBUF x PSUM -> SBUF)
     DMA: out tile (2 windows batched = 1 MiB) -> HBM.
"""
import os
import sys
from contextlib import ExitStack

import numpy as np

if "/opt/trn_rl_repo" not in sys.path:
    sys.path.insert(0, "/opt/trn_rl_repo")

import concourse.bass as bass
import concourse.bacc as bacc
import concourse.mybir as mybir
import concourse.tile as tile
from concourse import bass_utils

N, H, E = 1024, 64, 16
NCORES = 8
R = N // NCORES            # 128 rows per core
F32 = mybir.dt.float32
F32R = mybir.dt.float32r
AF = mybir.ActivationFunctionType
OP = mybir.AluOpType

NJ_CHUNK = 32              # j's per K-matmul chunk (lhsT partition rows)
NCHUNK = N // NJ_CHUNK     # 32 chunks
NJ_HALF = 16               # j's per PSUM window
WFREE = NJ_HALF * H        # 1024 free elements per window
NHALF = N // NJ_HALF       # 64 windows per core
WB = 4                     # windows per rhs/ahrow buffer + output DMA batch
OUT_BATCH = WB
F16 = mybir.dt.float16


def _sigmoid(x):
    return 1.0 / (1.0 + np.exp(-x))


def _fp32r(x):
    """Round fp32 -> fp32r (11 mantissa bits, round-half-even) like the PE."""
    b = np.ascontiguousarray(x, np.float32).view(np.uint32)
    rb = (b + np.uint32(0x7FF) + ((b >> np.uint32(12)) & np.uint32(1))) \
        & np.uint32(0xFFFFF000)
    return rb.view(np.float32)


def _host_prep(action_hidden_state, goal_hidden_state, goal, action,
               w_dist, b_dist, w_gate, b_gate):
    f32 = np.float32
    AH = np.ascontiguousarray(action_hidden_state, f32)
    GH = np.ascontiguousarray(goal_hidden_state, f32)
    goal = np.asarray(goal, f32)
    action = np.asarray(action, f32)
    w_dist = np.asarray(w_dist, f32)
    b_dist = np.asarray(b_dist, f32)
    w_gate = np.asarray(w_gate, f32)
    b_gate = np.asarray(b_gate, f32)

    ax, ay = action[:, 0].copy(), action[:, 1].copy()
    gx, gy = goal[:, 0].copy(), goal[:, 1].copy()
    gyx = gy - gx
    diagv = np.sqrt((ax - gx) ** 2 + (ay - gy) ** 2).astype(f32)

    W1, W2 = w_gate[0:H], w_gate[H:2 * H]
    W3, W4 = w_gate[2 * H:2 * H + E], w_gate[2 * H + E:2 * H + 2 * E]
    u3 = (w_dist @ W3).astype(f32)
    u4 = (w_dist @ W4).astype(f32)
    B = (AH @ (W1 + W2) + b_dist @ (W3 + W4) + b_gate).astype(f32)

    one = np.ones(N, f32)
    # rank factors: num[i,j] = sum_k f[k][i] * g[k][j]
    f_cav = np.stack([ax, -ax * gx, -ay, ay * gx])
    g_cav = np.stack([ay * gx, ay, ax * gx, ax])
    f_caz = np.stack([ax, -ax * gy, -ay, ay * gy])
    g_caz = np.stack([ay * gy, ay, ax * gy, ax])
    f_wcg1 = np.stack([gx, -ax * gx]); g_wcg1 = np.stack([ax * gyx, gyx])
    f_wcg2 = np.stack([gyx, -ax * gyx]); g_wcg2 = np.stack([ax * gx, gx])
    f_scg1 = np.stack([gx, -ay * gx]); g_scg1 = np.stack([ax * gyx, gyx])
    f_t2 = np.stack([gyx, -ax * gyx]); g_t2 = np.stack([ay * gx, gx])
    f_dnm = np.stack([one, -ay, -gx, ay * gx, np.zeros(N, f32), np.zeros(N, f32)])
    g_dnm = np.stack([ay * gx, gx, ay, one, np.zeros(N, f32), np.zeros(N, f32)])

    fg = dict(
        dnm=(f_dnm, g_dnm),
        num1=(np.concatenate([f_cav, -f_wcg1]), np.concatenate([g_cav, g_wcg1])),
        num1p=(np.concatenate([f_cav, f_wcg2]), np.concatenate([g_cav, g_wcg2])),
        num2=(np.concatenate([f_caz, -f_scg1]), np.concatenate([g_caz, g_scg1])),
        num2p=(np.concatenate([f_caz, f_t2]), np.concatenate([g_caz, g_t2])),
    )

    logit_diag = (B + (GH - AH) @ W2 + diagv[:, None] * (u3 + u4)).astype(f32)
    out_diag = (GH * _sigmoid(logit_diag)).astype(f32)

    # rhs18 static rows: 0..15 delta(j_local)*u4 over a 16-j window, 16 = u3;
    # tiled WB times (one buffer serves WB consecutive windows)
    rhs18s = np.zeros((17, WFREE), f32)
    for jl in range(NJ_HALF):
        rhs18s[jl, jl * H:(jl + 1) * H] = u4
    rhs18s[16] = np.tile(u3, NJ_HALF)
    rhs18s = _fp32r(np.tile(rhs18s, (1, WB)))          # [17, WB*WFREE]

    # per-window-group B rows (row 17 of rhs18) / AH rows (K=1 broadcast rhs)
    B_rows = _fp32r(B.reshape(NHALF // WB, WB * WFREE))
    AH_rows = AH.reshape(NHALF // WB, WB * WFREE).astype(np.float16)

    ones1 = np.ones((1, R), np.float16)

    return dict(AH=AH, GH=GH, ax=ax, ay=ay, gx=gx, gy=gy, diagv=diagv,
                u3=u3, u4=u4, B=B, fg=fg, out_diag=out_diag,
                B_rows=B_rows, AH_rows=AH_rows, rhs18s=rhs18s, ones1=ones1)


NUM_NAMES = ["dnm", "num1", "num1p", "num2", "num2p"]


def _core_inputs(prep, core):
    """Build the per-core in_map (numpy arrays for every ExternalInput)."""
    f32 = np.float32
    i0 = core * R
    sl = slice(i0, i0 + R)

    sc = np.zeros((R, 8), f32)
    sc[:, 0] = prep["ax"][sl]
    sc[:, 1] = prep["ay"][sl]
    sc[:, 2] = prep["gx"][sl]
    sc[:, 3] = prep["diagv"][sl]

    jj = np.arange(N)[None, :]
    ii = np.arange(i0, i0 + R)[:, None]
    mju = (jj > ii).astype(f32)
    mjl = (jj < ii).astype(f32)

    axj_b = np.broadcast_to(prep["ax"], (R, N)).copy()
    gxj_b = np.broadcast_to(prep["gx"], (R, N)).copy()

    ident = np.eye(128, dtype=f32)

    dvi_rep = _fp32r(np.tile(prep["diagv"][sl], NHALF)[None, :])  # [1, 64*128]
    ones_row = np.ones((1, NHALF * 128), f32)

    meye = (jj == ii).astype(f32)
    m = dict(sc=sc, mju=mju, mjl=mjl, meye=meye, axj_b=axj_b, gxj_b=gxj_b,
             ident=ident, dvi_rep=dvi_rep, ones_row=ones_row,
             rhs18s=prep["rhs18s"], ones1=prep["ones1"],
             B_rows=prep["B_rows"], AH_rows=prep["AH_rows"])
    for nm in NUM_NAMES:
        f, g = prep["fg"][nm]
        m[f"lhsT_{nm}"] = np.ascontiguousarray(f[:, sl].astype(f32))  # [6, 128]
        m[f"rhs_{nm}"] = np.ascontiguousarray(g.astype(f32))          # [6, 1024]
    return m


def _declare_tensors(nc):
    t = {}
    def inp(name, shape, dt=F32):
        t[name] = nc.dram_tensor(name, shape, dt, kind="ExternalInput").ap()
    inp("sc", [R, 8])
    inp("mju", [R, N]); inp("mjl", [R, N]); inp("meye", [R, N])
    inp("axj_b", [R, N]); inp("gxj_b", [R, N])
    inp("ident", [128, 128])
    inp("dvi_rep", [1, NHALF * 128], F32R)
    inp("ones_row", [1, NHALF * 128], F32R)
    inp("rhs18s", [17, WB * WFREE], F32R)
    inp("ones1", [1, R], F16)
    inp("B_rows", [NHALF // WB, WB * WFREE], F32R)
    inp("AH_rows", [NHALF // WB, WB * WFREE], F16)
    for nm in NUM_NAMES:
        inp(f"lhsT_{nm}", [6, 128])
        inp(f"rhs_{nm}", [6, N])
    t["out"] = nc.dram_tensor("out", [R, N * H], F32, kind="ExternalOutput").ap()
    return t


def _build_program(ctx, tc, t):
    nc = tc.nc

    consts = ctx.enter_context(tc.tile_pool(name="consts", bufs=1))

    def load_in(pool, name, shape, dt=F32):
        tl = pool.tile(shape, dt, tag=name, name=name)
        nc.sync.dma_start(tl[:], t[name])
        return tl

    def load(name, shape, dt=F32):
        return load_in(consts, name, shape, dt)

    sc = load("sc", [R, 8])
    ones1 = load("ones1", [1, R], F16)
    # persistent main-loop tiles allocated first (survive prologue pools)
    dT18 = consts.tile([18, NHALF * 128], F32R, tag="dT18", name="dT18")
    rhs18_bufs = []
    for bi in range(2):
        rb = consts.tile([18, WB * WFREE], F32R, tag=f"rhs18_{bi}",
                         name=f"rhs18_{bi}")
        nc.sync.dma_start(rb[0:17, :], t["rhs18s"])
        rhs18_bufs.append(rb)

    # prologue pool: everything phases 1-3 need; released before phase 4
    pro_cm = tc.tile_pool(name="pro", bufs=1)
    pro = pro_cm.__enter__()
    mju = load_in(pro, "mju", [R, N])
    mjl = load_in(pro, "mjl", [R, N])
    meye = load_in(pro, "meye", [R, N])
    axj_b = load_in(pro, "axj_b", [R, N])
    gxj_b = load_in(pro, "gxj_b", [R, N])
    ident = load_in(pro, "ident", [128, 128])
    lhsT_num = {nm: load_in(pro, f"lhsT_{nm}", [6, 128]) for nm in NUM_NAMES}
    rhs_num = {nm: load_in(pro, f"rhs_{nm}", [6, N]) for nm in NUM_NAMES}

    AXi, AYi, GXi, DVi = (sc[:, k:k + 1] for k in range(4))

    # ---- phase 1: numerators via PE (rank<=6), eviction to SBUF ----
    nums = pro
    work = pro
    num_sb = {}
    with tc.tile_pool(name="ps_num", bufs=2, space="PSUM") as ps_num:
        for nm in NUM_NAMES:
            ps = ps_num.tile([R, N], F32, tag="ps_num", name="ps_num")
            for w in range(N // 512):
                nc.tensor.matmul(ps[:, w * 512:(w + 1) * 512],
                                 lhsT_num[nm][:, :],
                                 rhs_num[nm][:, w * 512:(w + 1) * 512],
                                 start=True, stop=True)
            sb = nums.tile([R, N], F32, tag=f"num_{nm}", name=f"num_{nm}")
            nc.vector.tensor_copy(sb[:], ps[:])
            num_sb[nm] = sb

    # ---- phase 2: dist [128, 1024] elementwise ----
    def wtile():
        return work.tile([R, N], F32, tag="w", name="w", bufs=8)

    # p's overwrite their numerator tiles in place; rdn overwrites dnm
    rdn = num_sb["dnm"]
    nc.gpsimd.tensor_add(rdn[:], rdn[:], meye[:])
    nc.vector.reciprocal(rdn[:], rdn[:])
    p1, p2, p1p, p2p = (num_sb[k] for k in ("num1", "num2", "num1p", "num2p"))
    nc.vector.tensor_mul(p1[:], p1[:], rdn[:])
    nc.vector.tensor_mul(p2[:], p2[:], rdn[:])
    nc.vector.tensor_mul(p1p[:], p1p[:], rdn[:])
    nc.vector.tensor_mul(p2p[:], p2p[:], rdn[:])

    e1 = wtile()
    nc.vector.tensor_scalar(e1[:], p1[:], AXi, None, OP.subtract)
    q1 = wtile()
    nc.vector.scalar_tensor_tensor(q1[:], p1[:], GXi, e1[:], OP.subtract, OP.mult)
    e1s = wtile()
    nc.scalar.square(e1s[:], e1[:])
    e2 = e1  # e1 dead
    nc.vector.tensor_scalar(e2[:], p2[:], AYi, None, OP.subtract)
    e2s = p1  # p1 dead
    nc.scalar.square(e2s[:], e2[:])
    s12 = e2
    nc.vector.tensor_add(s12[:], e1s[:], e2s[:])
    d1p = wtile()
    nc.scalar.sqrt(d1p[:], s12[:])
    c1m = e1s
    nc.vector.tensor_scalar(c1m[:], q1[:], 0.0, None, OP.is_lt)
    m1 = q1
    nc.gpsimd.tensor_mul(m1[:], c1m[:], mju[:])

    g1 = s12
    nc.vector.tensor_scalar(g1[:], p1p[:], AXi, None, OP.subtract)
    g1s = c1m
    nc.scalar.square(g1s[:], g1[:])
    g2 = g1
    nc.vector.tensor_scalar(g2[:], p2p[:], AYi, None, OP.subtract)
    g2s = p2  # p2 dead
    nc.scalar.square(g2s[:], g2[:])
    s34 = g2
    nc.vector.tensor_add(s34[:], g1s[:], g2s[:])
    d2p = wtile()
    nc.scalar.sqrt(d2p[:], s34[:])

    t1 = g1s
    nc.gpsimd.tensor_sub(t1[:], p1p[:], axj_b[:])
    t2 = g2s
    nc.gpsimd.tensor_sub(t2[:], p1p[:], gxj_b[:])
    q2 = p1p  # p1p dead
    nc.gpsimd.tensor_mul(q2[:], t1[:], t2[:])
    c2m = t1
    nc.vector.tensor_scalar(c2m[:], q2[:], 0.0, None, OP.is_lt)
    m2 = t2
    nc.gpsimd.tensor_mul(m2[:], c2m[:], mjl[:])

    # walrus requires integer mask dtype for CopyPredicated
    mu1 = work.tile([R, N], mybir.dt.uint8, tag="mu1", name="mu1")
    mu2 = work.tile([R, N], mybir.dt.uint8, tag="mu2", name="mu2")
    nc.vector.tensor_copy(mu1[:], m1[:])
    nc.vector.tensor_copy(mu2[:], m2[:])

    dist = nums.tile([R, N], F32, tag="dist", name="dist")
    nc.vector.tensor_scalar(dist[:], mju[:], 0.0, DVi, OP.mult, OP.add)
    nc.vector.copy_predicated(dist[:], mu1[:], d1p[:])
    nc.vector.copy_predicated(dist[:], mu2[:], d2p[:])

    # ---- phase 3: transpose dist -> dT18 [18, NHALF*128] (float32r) ----
    # dT18[q, hh*128 + i] = dist[i, 16*hh + q]; row 16 = diagv rep, 17 = ones
    dT_sb = nums.tile([128, N], F32R, tag="dT_sb", name="dT_sb")
    with tc.tile_pool(name="ps_tr", bufs=2, space="PSUM") as ps_tr:
        for tt in range(8):
            ps = ps_tr.tile([128, 128], F32, tag="ps_tr", name="ps_tr")
            nc.tensor.transpose(ps[:], dist[:, tt * 128:(tt + 1) * 128], ident[:])
            nc.vector.tensor_copy(dT_sb[:, tt * 128:(tt + 1) * 128], ps[:])
    # relayout: dT18[q, (8g+b)*128 + i] = dT_sb[16b+q, 128g + i]
    dT18_v = dT18[0:16, :].rearrange("p (hh i) -> p hh i", i=128)
    dT_sb_v = dT_sb[:, :].rearrange("p (g i) -> p g i", i=128)
    for b in range(8):
        nc.sync.dma_start(dT18_v[:, b::8, :], dT_sb_v[16 * b:16 * (b + 1), :, :])
    nc.sync.dma_start(dT18[16:17, :], t["dvi_rep"])
    nc.sync.dma_start(dT18[17:18, :], t["ones_row"])

    pro_cm.__exit__(None, None, None)

    # ---- phase 4: main loop over 64 16-j windows, grouped by WB ----
    ps_logit = ctx.enter_context(tc.tile_pool(name="ps_logit", bufs=2, space="PSUM"))
    ps_ah = ctx.enter_context(tc.tile_pool(name="ps_ah", bufs=2, space="PSUM"))
    sig_pool = ctx.enter_context(tc.tile_pool(name="sig", bufs=3))
    out_pool = ctx.enter_context(tc.tile_pool(name="outsb", bufs=2))
    ahrow_pool = ctx.enter_context(tc.tile_pool(name="ahrow", bufs=2))

    out_sb = None
    ahrow = None
    for hh in range(NHALF):
        g, wi = hh // WB, hh % WB
        rb = rhs18_bufs[g % 2]
        if wi == 0:
            nc.sync.dma_start(rb[17:18, :], t["B_rows"][g:g + 1, :])
            ahrow = ahrow_pool.tile([1, WB * WFREE], F16, tag="ahrow",
                                    name="ahrow")
            nc.sync.dma_start(ahrow[:], t["AH_rows"][g:g + 1, :])
            out_sb = out_pool.tile([R, WB * WFREE], F32, tag="out_sb",
                                   name="out_sb")

        lg = ps_logit.tile([R, WFREE], F32, tag="lg", name="lg")
        ah = ps_ah.tile([R, WFREE], F32, tag="ah", name="ah")
        for w in range(2):
            dst = slice(w * 512, (w + 1) * 512)
            wsrc = slice(wi * WFREE + w * 512, wi * WFREE + (w + 1) * 512)
            nc.tensor.matmul(lg[:, dst], dT18[0:18, hh * 128:(hh + 1) * 128],
                             rb[:, wsrc], start=True, stop=True)
            nc.tensor.matmul(ah[:, dst], ones1[0:1, :], ahrow[0:1, wsrc],
                             start=True, stop=True)

        sig = sig_pool.tile([R, WFREE], F32, tag="sig", name="sig")
        nc.scalar.activation(sig[:], lg[:], AF.Sigmoid)

        seg = slice(wi * WFREE, (wi + 1) * WFREE)
        nc.vector.tensor_mul(out_sb[:, seg], sig[:], ah[:])
        if wi == WB - 1:
            base = (hh - (WB - 1)) * WFREE
            nc.sync.dma_start(t["out"][:, base:base + WB * WFREE], out_sb[:])


def build_nc():
    nc = bacc.Bacc("TRN2", target_bir_lowering=False, debug=False,
                   enable_asserts=False, num_devices=NCORES)
    t = _declare_tensors(nc)
    with tile.TileContext(nc) as tc:
        with ExitStack() as ctx:
            _build_program(ctx, tc, t)
    nc.compile()
    return nc


def kernel(**inputs):
    prep = _host_prep(**inputs)
    nc = build_nc()
    in_maps = [_core_inputs(prep, c) for c in range(NCORES)]
    res = bass_utils.run_bass_kernel_spmd(nc, in_maps, core_ids=list(range(NCORES)))
    out = np.concatenate([r["out"] for r in res.results], 0).reshape(N, N, H)
    # patch the diagonal (host-computed, uses GH and the diag logit)
    out[np.arange(N), np.arange(N)] = prep["out_diag"]
    return out


if __name__ == "__main__":
    import reference
    inputs = {k: np.asarray(v) for k, v in reference.setup_inputs().items()}
    out = kernel(**inputs)
    print("kernel out", out.shape, out.dtype)


# revision 31
# speedup vs baseline: 1.3795x; 1.1790x over previous
"""Trainium2 Bass kernel for nn_InteractionGate (gnn_message_passing).

Contract: kernel(**inputs) takes the FULL unsharded inputs (as in
reference.setup_inputs()) and returns the FULL [1024, 1024, 64] output.
Internally shards the pairwise row dimension i across 8 NeuronCores
(128 rows each), runs one SPMD Bass/Tile program on cores 0-7, gathers.

Math: with
  W1 = w_gate[0:64], W2 = w_gate[64:128], W3 = w_gate[128:144], W4 = w_gate[144:160]
  u3 = w_dist @ W3, u4 = w_dist @ W4
  B  = AH @ (W1+W2) + b_dist @ (W3+W4) + b_gate          [N,H]
the reference reduces (off-diagonal) to
  out[i,j,h] = AH[j,h] * sigmoid(B[j,h] + diagv[i]*u3[h] + dist[i,j]*u4[h])
where dist is the cal_dist "distance_other" matrix. The diagonal entries
use GH instead of AH and are patched on the host (O(N*H) work).

Device plan per core (rows i in its 128-block, partitions = i):
  1. PE computes the five pairwise numerator matrices (each is rank<=6:
     sum_k f_k(i) g_k(j)) as K=6 fp32 matmuls (cancellation-sensitive).
  2. DVE/ACT compute dist[i,j] [128,1024] elementwise (reciprocal, sqrt,
     branch masks via predicated copies).
  3. PE transposes dist into dT33 [33, 4096] (row 32 = diagv row) via 8
     128x128 transposes; PSUM evictions round to float32r.
  4. Main loop over 64 half-chunks (16 j's x 64 h = 1024 free each),
     all matmuls in float32r (1 PE cycle/column, 11-bit mantissa):
     PE:  logit  = dT33-chunk.T @ [delta*u4 ; u3row]  (K=33)
                 + ones.T @ B_row-slice               (K=1 broadcast)
          ah     = ones.T @ AH_row-slice              (K=1 broadcast)
     ACT: sig = sigmoid(logit)   (PSUM -> SBUF)
     DVE: out = sig * ah         (SBUF x PSUM -> SBUF)
     DMA: out tile (2 windows batched = 1 MiB) -> HBM.
"""
import os
import sys
from contextlib import ExitStack

import numpy as np

if "/opt/trn_rl_repo" not in sys.path:
    sys.path.insert(0, "/opt/trn_rl_repo")

import concourse.bass as bass
import concourse.bacc as bacc
import concourse.mybir as mybir
import concourse.tile as tile
from concourse import bass_utils

N, H, E = 1024, 64, 16
NCORES = 8
R = N // NCORES            # 128 rows per core
F32 = mybir.dt.float32
F32R = mybir.dt.float32r
AF = mybir.ActivationFunctionType
OP = mybir.AluOpType

NJ_CHUNK = 32              # j's per K-matmul chunk (lhsT partition rows)
NCHUNK = N // NJ_CHUNK     # 32 chunks
NJ_HALF = 16               # j's per PSUM window
WFREE = NJ_HALF * H        # 1024 free elements per window
NHALF = N // NJ_HALF       # 64 windows per core
WB = 4                     # windows per rhs/ahrow buffer + output DMA batch
OUT_BATCH = WB
F16 = mybir.dt.float16


def _sigmoid(x):
    return 1.0 / (1.0 + np.exp(-x))


def _fp32r(x):
    """Round fp32 -> fp32r (11 mantissa bits, round-half-even) like the PE."""
    b = np.ascontiguousarray(x, np.float32).view(np.uint32)
    rb = (b + np.uint32(0x7FF) + ((b >> np.uint32(12)) & np.uint32(1))) \
        & np.uint32(0xFFFFF000)
    return rb.view(np.float32)


def _host_prep(action_hidden_state, goal_hidden_state, goal, action,
               w_dist, b_dist, w_gate, b_gate):
    f32 = np.float32
    AH = np.ascontiguousarray(action_hidden_state, f32)
    GH = np.ascontiguousarray(goal_hidden_state, f32)
    goal = np.asarray(goal, f32)
    action = np.asarray(action, f32)
    w_dist = np.asarray(w_dist, f32)
    b_dist = np.asarray(b_dist, f32)
    w_gate = np.asarray(w_gate, f32)
    b_gate = np.asarray(b_gate, f32)

    ax, ay = action[:, 0].copy(), action[:, 1].copy()
    gx, gy = goal[:, 0].copy(), goal[:, 1].copy()
    gyx = gy - gx
    diagv = np.sqrt((ax - gx) ** 2 + (ay - gy) ** 2).astype(f32)

    W1, W2 = w_gate[0:H], w_gate[H:2 * H]
    W3, W4 = w_gate[2 * H:2 * H + E], w_gate[2 * H + E:2 * H + 2 * E]
    u3 = (w_dist @ W3).astype(f32)
    u4 = (w_dist @ W4).astype(f32)
    B = (AH @ (W1 + W2) + b_dist @ (W3 + W4) + b_gate).astype(f32)

    one = np.ones(N, f32)
    # rank factors: num[i,j] = sum_k f[k][i] * g[k][j]
    f_cav = np.stack([ax, -ax * gx, -ay, ay * gx])
    g_cav = np.stack([ay * gx, ay, ax * gx, ax])
    f_caz = np.stack([ax, -ax * gy, -ay, ay * gy])
    g_caz = np.stack([ay * gy, ay, ax * gy, ax])
    f_wcg1 = np.stack([gx, -ax * gx]); g_wcg1 = np.stack([ax * gyx, gyx])
    f_wcg2 = np.stack([gyx, -ax * gyx]); g_wcg2 = np.stack([ax * gx, gx])
    f_scg1 = np.stack([gx, -ay * gx]); g_scg1 = np.stack([ax * gyx, gyx])
    f_t2 = np.stack([gyx, -ax * gyx]); g_t2 = np.stack([ay * gx, gx])
    f_dnm = np.stack([one, -ay, -gx, ay * gx, np.zeros(N, f32), np.zeros(N, f32)])
    g_dnm = np.stack([ay * gx, gx, ay, one, np.zeros(N, f32), np.zeros(N, f32)])

    fg = dict(
        dnm=(f_dnm, g_dnm),
        num1=(np.concatenate([f_cav, -f_wcg1]), np.concatenate([g_cav, g_wcg1])),
        num1p=(np.concatenate([f_cav, f_wcg2]), np.concatenate([g_cav, g_wcg2])),
        num2=(np.concatenate([f_caz, -f_scg1]), np.concatenate([g_caz, g_scg1])),
        num2p=(np.concatenate([f_caz, f_t2]), np.concatenate([g_caz, g_t2])),
    )

    logit_diag = (B + (GH - AH) @ W2 + diagv[:, None] * (u3 + u4)).astype(f32)
    out_diag = (GH * _sigmoid(logit_diag)).astype(f32)

    # rhs18 static rows: 0..15 delta(j_local)*u4 over a 16-j window, 16 = u3;
    # tiled WB times (one buffer serves WB consecutive windows)
    rhs18s = np.zeros((17, WFREE), f32)
    for jl in range(NJ_HALF):
        rhs18s[jl, jl * H:(jl + 1) * H] = u4
    rhs18s[16] = np.tile(u3, NJ_HALF)
    rhs18s = np.tile(rhs18s, (1, WB)).astype(np.float16)   # [17, WB*WFREE]

    # per-window-group B rows (row 17 of rhs18) / AH rows (K=1 broadcast rhs)
    B_rows = B.reshape(NHALF // WB, WB * WFREE).astype(np.float16)
    AH_rows = AH.reshape(NHALF // WB, WB * WFREE).astype(np.float16)

    ones1 = np.ones((1, R), np.float16)

    return dict(AH=AH, GH=GH, ax=ax, ay=ay, gx=gx, gy=gy, diagv=diagv,
                u3=u3, u4=u4, B=B, fg=fg, out_diag=out_diag,
                B_rows=B_rows, AH_rows=AH_rows, rhs18s=rhs18s, ones1=ones1)


NUM_NAMES = ["dnm", "num1", "num1p", "num2", "num2p"]


def _core_inputs(prep, core):
    """Build the per-core in_map (numpy arrays for every ExternalInput)."""
    f32 = np.float32
    i0 = core * R
    sl = slice(i0, i0 + R)

    sc = np.zeros((R, 8), f32)
    sc[:, 0] = prep["ax"][sl]
    sc[:, 1] = prep["ay"][sl]
    sc[:, 2] = prep["gx"][sl]
    sc[:, 3] = prep["diagv"][sl]

    jj = np.arange(N)[None, :]
    ii = np.arange(i0, i0 + R)[:, None]
    mju = (jj > ii).astype(f32)
    mjl = (jj < ii).astype(f32)

    axj_b = np.broadcast_to(prep["ax"], (R, N)).copy()
    gxj_b = np.broadcast_to(prep["gx"], (R, N)).copy()

    ident = np.eye(128, dtype=f32)

    dvi_rep = np.tile(prep["diagv"][sl], NHALF)[None, :].astype(np.float16)
    ones_row = np.ones((1, NHALF * 128), np.float16)

    meye = (jj == ii).astype(f32)
    m = dict(sc=sc, mju=mju, mjl=mjl, meye=meye, axj_b=axj_b, gxj_b=gxj_b,
             ident=ident, dvi_rep=dvi_rep, ones_row=ones_row,
             rhs18s=prep["rhs18s"], ones1=prep["ones1"],
             B_rows=prep["B_rows"], AH_rows=prep["AH_rows"])
    for nm in NUM_NAMES:
        f, g = prep["fg"][nm]
        m[f"lhsT_{nm}"] = np.ascontiguousarray(f[:, sl].astype(f32))  # [6, 128]
        m[f"rhs_{nm}"] = np.ascontiguousarray(g.astype(f32))          # [6, 1024]
    return m


def _declare_tensors(nc):
    t = {}
    def inp(name, shape, dt=F32):
        t[name] = nc.dram_tensor(name, shape, dt, kind="ExternalInput").ap()
    inp("sc", [R, 8])
    inp("mju", [R, N]); inp("mjl", [R, N]); inp("meye", [R, N])
    inp("axj_b", [R, N]); inp("gxj_b", [R, N])
    inp("ident", [128, 128])
    inp("dvi_rep", [1, NHALF * 128], F16)
    inp("ones_row", [1, NHALF * 128], F16)
    inp("rhs18s", [17, WB * WFREE], F16)
    inp("ones1", [1, R], F16)
    inp("B_rows", [NHALF // WB, WB * WFREE], F16)
    inp("AH_rows", [NHALF // WB, WB * WFREE], F16)
    for nm in NUM_NAMES:
        inp(f"lhsT_{nm}", [6, 128])
        inp(f"rhs_{nm}", [6, N])
    t["out"] = nc.dram_tensor("out", [R, N * H], F32, kind="ExternalOutput").ap()
    return t


def _build_program(ctx, tc, t):
    nc = tc.nc

    consts = ctx.enter_context(tc.tile_pool(name="consts", bufs=1))

    def load_in(pool, name, shape, dt=F32):
        tl = pool.tile(shape, dt, tag=name, name=name)
        nc.sync.dma_start(tl[:], t[name])
        return tl

    def load(name, shape, dt=F32):
        return load_in(consts, name, shape, dt)

    sc = load("sc", [R, 8])
    ones1 = load("ones1", [1, R], F16)
    # persistent main-loop tiles allocated first (survive prologue pools)
    dT18 = consts.tile([18, NHALF * 128], F16, tag="dT18", name="dT18")
    rhs18_bufs = []
    ahrow_bufs = []
    for bi in range(3):
        rb = consts.tile([18, WB * WFREE], F16, tag=f"rhs18_{bi}",
                         name=f"rhs18_{bi}")
        rhs18_bufs.append(rb)
        ab = consts.tile([1, WB * WFREE], F16, tag=f"ahrow_{bi}",
                         name=f"ahrow_{bi}")
        ahrow_bufs.append(ab)

    # prologue pool: everything phases 1-3 need; released before phase 4
    pro_cm = tc.tile_pool(name="pro", bufs=1)
    pro = pro_cm.__enter__()
    # phase-1 inputs first: they gate the numerator matmuls
    lhsT_num = {nm: load_in(pro, f"lhsT_{nm}", [6, 128]) for nm in NUM_NAMES}
    rhs_num = {nm: load_in(pro, f"rhs_{nm}", [6, N]) for nm in NUM_NAMES}
    ident = load_in(pro, "ident", [128, 128])
    meye = load_in(pro, "meye", [R, N])
    mju = load_in(pro, "mju", [R, N])
    mjl = load_in(pro, "mjl", [R, N])
    axj_b = load_in(pro, "axj_b", [R, N])
    gxj_b = load_in(pro, "gxj_b", [R, N])
    for bi in range(3):
        nc.sync.dma_start(rhs18_bufs[bi][0:17, :], t["rhs18s"])

    AXi, AYi, GXi, DVi = (sc[:, k:k + 1] for k in range(4))

    # ---- phase 1: numerators via PE (rank<=6), eviction to SBUF ----
    nums = pro
    work = pro
    num_sb = {}
    with tc.tile_pool(name="ps_num", bufs=2, space="PSUM") as ps_num:
        for nm in NUM_NAMES:
            ps = ps_num.tile([R, N], F32, tag="ps_num", name="ps_num")
            for w in range(N // 512):
                nc.tensor.matmul(ps[:, w * 512:(w + 1) * 512],
                                 lhsT_num[nm][:, :],
                                 rhs_num[nm][:, w * 512:(w + 1) * 512],
                                 start=True, stop=True)
            sb = nums.tile([R, N], F32, tag=f"num_{nm}", name=f"num_{nm}")
            nc.vector.tensor_copy(sb[:], ps[:])
            num_sb[nm] = sb

    # ---- phase 2: dist [128, 1024] elementwise ----
    def wtile():
        return work.tile([R, N], F32, tag="w", name="w", bufs=8)

    # p's overwrite their numerator tiles in place; rdn overwrites dnm
    rdn = num_sb["dnm"]
    nc.vector.tensor_add(rdn[:], rdn[:], meye[:])
    nc.vector.reciprocal(rdn[:], rdn[:])
    p1, p2, p1p, p2p = (num_sb[k] for k in ("num1", "num2", "num1p", "num2p"))
    nc.vector.tensor_mul(p1[:], p1[:], rdn[:])
    nc.vector.tensor_mul(p2[:], p2[:], rdn[:])
    nc.vector.tensor_mul(p1p[:], p1p[:], rdn[:])
    nc.vector.tensor_mul(p2p[:], p2p[:], rdn[:])

    e1 = wtile()
    nc.vector.tensor_scalar(e1[:], p1[:], AXi, None, OP.subtract)
    q1 = wtile()
    nc.vector.scalar_tensor_tensor(q1[:], p1[:], GXi, e1[:], OP.subtract, OP.mult)
    e1s = wtile()
    nc.scalar.square(e1s[:], e1[:])
    e2 = e1  # e1 dead
    nc.vector.tensor_scalar(e2[:], p2[:], AYi, None, OP.subtract)
    e2s = p1  # p1 dead
    nc.scalar.square(e2s[:], e2[:])
    s12 = e2
    nc.vector.tensor_add(s12[:], e1s[:], e2s[:])
    d1p = wtile()
    nc.scalar.sqrt(d1p[:], s12[:])
    # mask: (q1<0) & (j>i)  ==  (q1*mju < 0)   (mju is 0/1)
    m1 = e1s
    nc.vector.tensor_mul(m1[:], q1[:], mju[:])

    g1 = s12
    nc.vector.tensor_scalar(g1[:], p1p[:], AXi, None, OP.subtract)
    g1s = q1
    nc.scalar.square(g1s[:], g1[:])
    g2 = g1
    nc.vector.tensor_scalar(g2[:], p2p[:], AYi, None, OP.subtract)
    g2s = p2  # p2 dead
    nc.scalar.square(g2s[:], g2[:])
    s34 = g2
    nc.vector.tensor_add(s34[:], g1s[:], g2s[:])
    d2p = wtile()
    nc.scalar.sqrt(d2p[:], s34[:])

    t1 = g1s
    nc.vector.tensor_sub(t1[:], p1p[:], axj_b[:])
    t2 = g2s
    nc.vector.tensor_sub(t2[:], p1p[:], gxj_b[:])
    q2 = p1p  # p1p dead
    nc.vector.tensor_mul(q2[:], t1[:], t2[:])
    m2 = t1
    nc.vector.tensor_mul(m2[:], q2[:], mjl[:])

    # walrus requires integer mask dtype for CopyPredicated
    mu1 = work.tile([R, N], mybir.dt.uint8, tag="mu1", name="mu1")
    mu2 = work.tile([R, N], mybir.dt.uint8, tag="mu2", name="mu2")
    nc.vector.tensor_scalar(mu1[:], m1[:], 0.0, None, OP.is_lt)
    nc.vector.tensor_scalar(mu2[:], m2[:], 0.0, None, OP.is_lt)

    dist = nums.tile([R, N], F32, tag="dist", name="dist")
    nc.vector.tensor_scalar(dist[:], mju[:], 0.0, DVi, OP.mult, OP.add)
    nc.vector.copy_predicated(dist[:], mu1[:], d1p[:])
    nc.vector.copy_predicated(dist[:], mu2[:], d2p[:])

    # ---- phase 3: transpose dist -> dT18 [18, NHALF*128] (float32r) ----
    # dT18[q, hh*128 + i] = dist[i, 16*hh + q]; row 16 = diagv rep, 17 = ones
    dT_sb = nums.tile([128, N], F16, tag="dT_sb", name="dT_sb")
    with tc.tile_pool(name="ps_tr", bufs=2, space="PSUM") as ps_tr:
        for tt in range(8):
            ps = ps_tr.tile([128, 128], F32, tag="ps_tr", name="ps_tr")
            nc.tensor.transpose(ps[:], dist[:, tt * 128:(tt + 1) * 128], ident[:])
            nc.vector.tensor_copy(dT_sb[:, tt * 128:(tt + 1) * 128], ps[:])
    # relayout: dT18[q, (8g+b)*128 + i] = dT_sb[16b+q, 128g + i]
    dT18_v = dT18[0:16, :].rearrange("p (hh i) -> p hh i", i=128)
    dT_sb_v = dT_sb[:, :].rearrange("p (g i) -> p g i", i=128)
    for b in range(8):
        nc.sync.dma_start(dT18_v[:, b::8, :], dT_sb_v[16 * b:16 * (b + 1), :, :])
    nc.sync.dma_start(dT18[16:17, :], t["dvi_rep"])
    nc.sync.dma_start(dT18[17:18, :], t["ones_row"])

    pro_cm.__exit__(None, None, None)

    # ---- phase 4: main loop over 64 16-j windows, grouped by WB ----
    NG = NHALF // WB
    ps_logit = ctx.enter_context(tc.tile_pool(name="ps_logit", bufs=2, space="PSUM"))
    ps_ah = ctx.enter_context(tc.tile_pool(name="ps_ah", bufs=2, space="PSUM"))
    sig_pool = ctx.enter_context(tc.tile_pool(name="sig", bufs=3))
    out_pool = ctx.enter_context(tc.tile_pool(name="outsb", bufs=3))

    def fetch_group(g):
        nc.sync.dma_start(rhs18_bufs[g % 3][17:18, :], t["B_rows"][g:g + 1, :])
        nc.sync.dma_start(ahrow_bufs[g % 3][:], t["AH_rows"][g:g + 1, :])

    fetch_group(0)
    fetch_group(1)
    out_sb = None
    for hh in range(NHALF):
        g, wi = hh // WB, hh % WB
        rb = rhs18_bufs[g % 3]
        ahrow = ahrow_bufs[g % 3]
        if wi == 0:
            out_sb = out_pool.tile([R, WB * WFREE], F32, tag="out_sb",
                                   name="out_sb")
            if g + 2 < NG:
                fetch_group(g + 2)

        lg = ps_logit.tile([R, WFREE], F32, tag="lg", name="lg")
        ah = ps_ah.tile([R, WFREE], F32, tag="ah", name="ah")
        for w in range(2):
            dst = slice(w * 512, (w + 1) * 512)
            wsrc = slice(wi * WFREE + w * 512, wi * WFREE + (w + 1) * 512)
            nc.tensor.matmul(lg[:, dst], dT18[0:18, hh * 128:(hh + 1) * 128],
                             rb[:, wsrc], start=True, stop=True)
            nc.tensor.matmul(ah[:, dst], ones1[0:1, :], ahrow[0:1, wsrc],
                             start=True, stop=True)

        sig = sig_pool.tile([R, WFREE], F32, tag="sig", name="sig")
        nc.scalar.activation(sig[:], lg[:], AF.Sigmoid)

        seg = slice(wi * WFREE, (wi + 1) * WFREE)
        nc.vector.tensor_mul(out_sb[:, seg], sig[:], ah[:])
        if wi == WB - 1:
            base = (hh - (WB - 1)) * WFREE
            nc.sync.dma_start(t["out"][:, base:base + WB * WFREE], out_sb[:])


def build_nc():
    nc = bacc.Bacc("TRN2", target_bir_lowering=False, debug=False,
                   enable_asserts=False, num_devices=NCORES)
    t = _declare_tensors(nc)
    with tile.TileContext(nc) as tc:
        with ExitStack() as ctx:
            _build_program(ctx, tc, t)
    nc.compile()
    return nc


def kernel(**inputs):
    prep = _host_prep(**inputs)
    nc = build_nc()
    in_maps = [_core_inputs(prep, c) for c in range(NCORES)]
    res = bass_utils.run_bass_kernel_spmd(nc, in_maps, core_ids=list(range(NCORES)))
    out = np.concatenate([r["out"] for r in res.results], 0).reshape(N, N, H)
    # patch the diagonal (host-computed, uses GH and the diag logit)
    out[np.arange(N), np.arange(N)] = prep["out_diag"]
    return out


if __name__ == "__main__":
    import reference
    inputs = {k: np.asarray(v) for k, v in reference.setup_inputs().items()}
    out = kernel(**inputs)
    print("kernel out", out.shape, out.dtype)


# revision 60
# speedup vs baseline: 1.8033x; 1.3072x over previous
"""Trainium2 Bass kernel for nn_InteractionGate (gnn_message_passing).

Contract: kernel(**inputs) takes the FULL unsharded inputs (as in
reference.setup_inputs()) and returns the FULL [1024, 1024, 64] output.
Internally shards the pairwise row dimension i across 8 NeuronCores
(128 rows each), runs one SPMD Bass/Tile program on cores 0-7, gathers.

Math: with
  W1 = w_gate[0:64], W2 = w_gate[64:128], W3 = w_gate[128:144], W4 = w_gate[144:160]
  u3 = w_dist @ W3, u4 = w_dist @ W4
  B  = AH @ (W1+W2) + b_dist @ (W3+W4) + b_gate          [N,H]
the reference reduces (off-diagonal) to
  out[i,j,h] = AH[j,h] * sigmoid(B[j,h] + diagv[i]*u3[h] + dist[i,j]*u4[h])
where dist is the cal_dist "distance_other" matrix. The diagonal entries
use GH instead of AH and are patched on the host (O(N*H) work).

Device plan per core (rows i in its 128-block, partitions = i):
  1. PE computes the five pairwise numerator matrices (each is rank<=6:
     sum_k f_k(i) g_k(j)) as K=6 fp32 matmuls (cancellation-sensitive).
  2. DVE/ACT/Pool compute dist[i,j] [128,1024] elementwise in two j-halves
     (fp32 for the cancellation-sensitive intersection points, fp16 for the
     distance chain), the second half interleaved under the main loop.
  3. DMA-XBAR transposes dist (fp16) into dT_sb [128, 1024]
     (dT_sb[p, t*128+i] = dist[i, 128t+p]; no PE/PSUM), then relayout
     DMAs build dT18 [18, 64*128]: dT18[q, hh*128+i] = dist[i, 16hh+q],
     row 16 = diagv rep, row 17 = ones.
  4. Main loop over 64 16-j windows (1024 free cols, all-fp16 PE):
     PE:  logit = dT18[:, hh*128:+128].T @ rhs18  (K=18: u4-delta rows,
          u3 row, B row -- B DMA'd into row 17 per 4-window group)
     ACT: sig = sigmoid(logit)  (PSUM -> SBUF fp16; lg pool bufs=4)
     DVE: out16 = sig * ahb     (all-fp16 SBUF, 2x mode)
     ahb = AH rows broadcast across partitions by DMA (per group)
     DMA: out fp16 (512 KiB per group) -> HBM; host upcasts to fp32.
"""
import os
import sys
from collections import deque
from contextlib import ExitStack

import numpy as np

if "/opt/trn_rl_repo" not in sys.path:
    sys.path.insert(0, "/opt/trn_rl_repo")

import concourse.bass as bass
import concourse.bacc as bacc
import concourse.mybir as mybir
import concourse.tile as tile
from concourse import bass_utils

N, H, E = 1024, 64, 16
NCORES = 8
R = N // NCORES            # 128 rows per core
F32 = mybir.dt.float32
F16 = mybir.dt.float16
U8 = mybir.dt.uint8
BF16 = mybir.dt.bfloat16
AF = mybir.ActivationFunctionType
OP = mybir.AluOpType

NJW = 16                   # j's per window
WFREE = NJW * H            # 1024 free elements per window
NW = N // NJW              # 64 windows
WB = 4                     # windows per group (buffers + out DMA batch)
NG = NW // WB              # 16 groups


def _sigmoid(x):
    return 1.0 / (1.0 + np.exp(-x))


def jnp_bf16(x):
    import ml_dtypes
    return np.asarray(x, np.float32).astype(ml_dtypes.bfloat16)


def _host_prep(action_hidden_state, goal_hidden_state, goal, action,
               w_dist, b_dist, w_gate, b_gate):
    f32, f16 = np.float32, np.float16
    AH = np.ascontiguousarray(action_hidden_state, f32)
    GH = np.ascontiguousarray(goal_hidden_state, f32)
    goal = np.asarray(goal, f32)
    action = np.asarray(action, f32)
    w_dist = np.asarray(w_dist, f32)
    b_dist = np.asarray(b_dist, f32)
    w_gate = np.asarray(w_gate, f32)
    b_gate = np.asarray(b_gate, f32)

    ax, ay = action[:, 0].copy(), action[:, 1].copy()
    gx, gy = goal[:, 0].copy(), goal[:, 1].copy()
    gyx = gy - gx
    diagv = np.sqrt((ax - gx) ** 2 + (ay - gy) ** 2).astype(f32)

    W1, W2 = w_gate[0:H], w_gate[H:2 * H]
    W3, W4 = w_gate[2 * H:2 * H + E], w_gate[2 * H + E:2 * H + 2 * E]
    u3 = (w_dist @ W3).astype(f32)
    u4 = (w_dist @ W4).astype(f32)
    B = (AH @ (W1 + W2) + b_dist @ (W3 + W4) + b_gate).astype(f32)

    one = np.ones(N, f32)
    # rank factors: num[i,j] = sum_k f[k][i] * g[k][j]
    f_cav = np.stack([ax, -ax * gx, -ay, ay * gx])
    g_cav = np.stack([ay * gx, ay, ax * gx, ax])
    f_caz = np.stack([ax, -ax * gy, -ay, ay * gy])
    g_caz = np.stack([ay * gy, ay, ax * gy, ax])
    f_wcg1 = np.stack([gx, -ax * gx]); g_wcg1 = np.stack([ax * gyx, gyx])
    f_wcg2 = np.stack([gyx, -ax * gyx]); g_wcg2 = np.stack([ax * gx, gx])
    f_scg1 = np.stack([gx, -ay * gx]); g_scg1 = np.stack([ax * gyx, gyx])
    f_t2 = np.stack([gyx, -ax * gyx]); g_t2 = np.stack([ay * gx, gx])
    f_dnm = np.stack([one, -ay, -gx, ay * gx, np.zeros(N, f32), np.zeros(N, f32)])
    g_dnm = np.stack([ay * gx, gx, ay, one, np.zeros(N, f32), np.zeros(N, f32)])

    fg = dict(
        dnm=(f_dnm, g_dnm),
        num1=(np.concatenate([f_cav, -f_wcg1]), np.concatenate([g_cav, g_wcg1])),
        num1p=(np.concatenate([f_cav, f_wcg2]), np.concatenate([g_cav, g_wcg2])),
        num2=(np.concatenate([f_caz, -f_scg1]), np.concatenate([g_caz, g_scg1])),
        num2p=(np.concatenate([f_caz, f_t2]), np.concatenate([g_caz, g_t2])),
    )

    logit_diag = (B + (GH - AH) @ W2 + diagv[:, None] * (u3 + u4)).astype(f32)
    out_diag = (GH * _sigmoid(logit_diag)).astype(f32)

    # rhs18 static rows 0..16: u4-delta for a 16-j window + u3 row, tiled WB x
    rhs18s = np.zeros((17, WFREE), f32)
    for q in range(NJW):
        rhs18s[q, q * H:(q + 1) * H] = u4
    rhs18s[16] = np.tile(u3, NJW)
    rhs18s = np.tile(rhs18s, (1, WB)).astype(f16)           # [17, WB*WFREE]

    B_rows = B.reshape(NG, WB * WFREE).astype(f16)          # [16, 4096]
    # AH scaled by 256 so out*256 stays in fp16 normal range (the host
    # divides it back out); kills the fp16-subnormal precision cliff.
    AH_rows = (AH.reshape(NG, WB * WFREE) * 256.0).astype(f16)  # [16, 4096]

    return dict(AH=AH, GH=GH, ax=ax, ay=ay, gx=gx, gy=gy, diagv=diagv,
                u3=u3, u4=u4, B=B, fg=fg, out_diag=out_diag,
                rhs18s=rhs18s, B_rows=B_rows, AH_rows=AH_rows)


NUM_NAMES = ["dnm", "num1", "num1p", "num2", "num2p"]


def _core_inputs(prep, core):
    """Build the per-core in_map (numpy arrays for every ExternalInput)."""
    f32, f16 = np.float32, np.float16
    i0 = core * R
    sl = slice(i0, i0 + R)

    sc = np.zeros((R, 8), f32)
    sc[:, 0] = prep["ax"][sl]
    sc[:, 1] = prep["ay"][sl]
    sc[:, 2] = prep["gx"][sl]
    sc[:, 3] = prep["diagv"][sl]

    jj = np.arange(N)[None, :]
    ii = np.arange(i0, i0 + R)[:, None]
    mju = (jj > ii).astype(f32)
    mjl = (jj < ii).astype(f32)
    meye = (jj == ii).astype(f32)

    axj_b = np.broadcast_to(prep["ax"], (R, N)).copy()
    gxj_b = np.broadcast_to(prep["gx"], (R, N)).copy()

    dvi_rep = np.tile(prep["diagv"][sl], NW)[None, :].astype(f16)  # [1, 64*128]
    ones_row = np.ones((1, NW * 128), f16)

    m = dict(sc=sc, mju=mju, mjl=mjl, meye=meye, axj_b=axj_b, gxj_b=gxj_b,
             dvi_rep=dvi_rep, ones_row=ones_row, rhs18s=prep["rhs18s"],
             B_rows=prep["B_rows"], AH_rows=prep["AH_rows"])
    for nm in NUM_NAMES:
        f, g = prep["fg"][nm]
        m[f"lhsT_{nm}"] = np.ascontiguousarray(f[:, sl].astype(f32))  # [6, 128]
        m[f"rhs_{nm}"] = np.ascontiguousarray(g.astype(f32))          # [6, 1024]
    return m


def _declare_tensors(nc):
    t = {}
    def inp(name, shape, dt=F32):
        t[name] = nc.dram_tensor(name, shape, dt, kind="ExternalInput").ap()
    inp("sc", [R, 8])
    inp("mju", [R, N]); inp("mjl", [R, N]); inp("meye", [R, N])
    inp("axj_b", [R, N]); inp("gxj_b", [R, N])
    inp("dvi_rep", [1, NW * 128], F16)
    inp("ones_row", [1, NW * 128], F16)
    inp("rhs18s", [17, WB * WFREE], F16)
    inp("B_rows", [NG, WB * WFREE], F16)
    inp("AH_rows", [NG, WB * WFREE], F16)
    for nm in NUM_NAMES:
        inp(f"lhsT_{nm}", [6, 128])
        inp(f"rhs_{nm}", [6, N])
    t["out"] = nc.dram_tensor("out", [R, N * H], F16, kind="ExternalOutput").ap()
    if os.environ.get("KDBG"):
        t["dbg_dist"] = nc.dram_tensor("dbg_dist", [R, N], F16, kind="ExternalOutput").ap()
        t["dbg_dT18"] = nc.dram_tensor("dbg_dT18", [18, NW * 128], F16, kind="ExternalOutput").ap()
    return t


def _build_program(ctx, tc, t):
    nc = tc.nc

    consts = ctx.enter_context(tc.tile_pool(name="consts", bufs=1))

    def load_in(pool, name, shape, dt=F32):
        tl = pool.tile(shape, dt, tag=name, name=name)
        nc.sync.dma_start(tl[:], t[name])
        return tl

    sc = load_in(consts, "sc", [R, 8])
    dT18 = consts.tile([18, NW * 128], F16, tag="dT18", name="dT18")
    dT_sb = consts.tile([128, N], F16, tag="dT_sb", name="dT_sb")
    rhs18_bufs = []
    ahb_bufs = []
    for bi in range(3):
        rb = consts.tile([18, WB * WFREE], F16, tag=f"rhs18_{bi}",
                         name=f"rhs18_{bi}")
        nc.sync.dma_start(rb[0:17, :], t["rhs18s"])
        rhs18_bufs.append(rb)
        ab = consts.tile([128, WB * WFREE], F16, tag=f"ahb_{bi}",
                         name=f"ahb_{bi}")
        ahb_bufs.append(ab)

    # prologue pool: everything phases 1-3 need
    pro = ctx.enter_context(tc.tile_pool(name="pro", bufs=1))
    lhsT_num = {nm: load_in(pro, f"lhsT_{nm}", [6, 128]) for nm in NUM_NAMES}
    rhs_num = {nm: load_in(pro, f"rhs_{nm}", [6, N]) for nm in NUM_NAMES}
    meye = load_in(pro, "meye", [R, N])
    mju = load_in(pro, "mju", [R, N])
    mjl = load_in(pro, "mjl", [R, N])
    axj_b = load_in(pro, "axj_b", [R, N])
    gxj_b = load_in(pro, "gxj_b", [R, N])

    AXi, AYi, GXi, DVi = (sc[:, k:k + 1] for k in range(4))

    # ---- phase 1: numerators via PE (rank<=6, fp32), eviction to SBUF ----
    num_sb = {}
    ps_num_cm = tc.tile_pool(name="ps_num", bufs=2, space="PSUM")
    ps_num = ps_num_cm.__enter__()
    for nm in NUM_NAMES:
        ps = ps_num.tile([R, N], F32, tag="ps_num", name="ps_num")
        for w in range(N // 512):
            nc.tensor.matmul(ps[:, w * 512:(w + 1) * 512],
                             lhsT_num[nm][:, :],
                             rhs_num[nm][:, w * 512:(w + 1) * 512],
                             start=True, stop=True)
        sb = pro.tile([R, N], F32, tag=f"num_{nm}", name=f"num_{nm}")
        nc.scalar.copy(sb[:], ps[:])
        num_sb[nm] = sb
    ps_num_cm.__exit__(None, None, None)

    # ---- phase 2: dist [128, cs] elementwise, per column-slice ----
    p1, p2, p1p, p2p = (num_sb[k] for k in ("num1", "num2", "num1p", "num2p"))
    rdn = num_sb["dnm"]

    def mk(tag, dt):
        return pro.tile([R, N], dt, tag=tag, name=tag)

    e1 = mk("e1", F32); q1 = mk("q1", F32)
    e1s = mk("e1s", F32); e2s = mk("e2s", F32)
    s12 = mk("s12", F32); d1p = mk("d1p", F16); d2p = mk("d2p", F16)
    t1 = mk("t1", F32); t2 = mk("t2", F32)
    m1 = mk("m1", F32)
    q2 = q1; m2 = m1  # branch-2 aliases (branch-1 uses precede)
    mu1 = mk("mu1", U8); mu2 = mk("mu2", U8)
    dist = mk("dist", F16)

    def ph2_thunks(cs):
        """Elementwise dist computation on column slice cs, as thunk list."""
        th = []
        # intersection points (fp32, cancellation-sensitive)
        th.append(lambda: nc.vector.tensor_add(rdn[:, cs], rdn[:, cs], meye[:, cs]))
        th.append(lambda: nc.vector.reciprocal(rdn[:, cs], rdn[:, cs]))
        th.append(lambda: nc.vector.tensor_mul(p1[:, cs], p1[:, cs], rdn[:, cs]))
        th.append(lambda: nc.vector.tensor_mul(p2[:, cs], p2[:, cs], rdn[:, cs]))
        th.append(lambda: nc.gpsimd.tensor_mul(p1p[:, cs], p1p[:, cs], rdn[:, cs]))
        th.append(lambda: nc.gpsimd.tensor_mul(p2p[:, cs], p2p[:, cs], rdn[:, cs]))
        # upper-triangle branch: d1p = |a_i - p|, cond (ax-p1)(p1-gx)>0
        th.append(lambda: nc.vector.tensor_scalar(e1[:, cs], p1[:, cs], AXi, None, OP.subtract))
        th.append(lambda: nc.vector.scalar_tensor_tensor(q1[:, cs], p1[:, cs], GXi, e1[:, cs], OP.subtract, OP.mult))
        th.append(lambda: nc.scalar.square(e1s[:, cs], e1[:, cs]))
        th.append(lambda: nc.vector.tensor_scalar(e1[:, cs], p2[:, cs], AYi, None, OP.subtract))
        th.append(lambda: nc.scalar.square(e2s[:, cs], e1[:, cs]))
        th.append(lambda: nc.vector.tensor_add(s12[:, cs], e1s[:, cs], e2s[:, cs]))
        th.append(lambda: nc.scalar.sqrt(d1p[:, cs], s12[:, cs]))
        th.append(lambda: nc.vector.tensor_scalar(m1[:, cs], q1[:, cs], 0.0, None, OP.is_lt))
        th.append(lambda: nc.gpsimd.tensor_mul(m1[:, cs], m1[:, cs], mju[:, cs]))
        th.append(lambda: nc.vector.tensor_copy(mu1[:, cs], m1[:, cs]))
        # lower-triangle branch: d2p = |a_i - p'|, cond (axj-p1')(p1'-gxj)>0
        th.append(lambda: nc.vector.tensor_scalar(e1[:, cs], p1p[:, cs], AXi, None, OP.subtract))
        th.append(lambda: nc.scalar.square(e1s[:, cs], e1[:, cs]))
        th.append(lambda: nc.vector.tensor_scalar(e1[:, cs], p2p[:, cs], AYi, None, OP.subtract))
        th.append(lambda: nc.scalar.square(e2s[:, cs], e1[:, cs]))
        th.append(lambda: nc.vector.tensor_add(s12[:, cs], e1s[:, cs], e2s[:, cs]))
        th.append(lambda: nc.scalar.sqrt(d2p[:, cs], s12[:, cs]))
        th.append(lambda: nc.vector.tensor_sub(t1[:, cs], p1p[:, cs], axj_b[:, cs]))
        th.append(lambda: nc.gpsimd.tensor_sub(t2[:, cs], p1p[:, cs], gxj_b[:, cs]))
        th.append(lambda: nc.vector.tensor_mul(q2[:, cs], t1[:, cs], t2[:, cs]))
        th.append(lambda: nc.vector.tensor_scalar(m2[:, cs], q2[:, cs], 0.0, None, OP.is_lt))
        th.append(lambda: nc.gpsimd.tensor_mul(m2[:, cs], m2[:, cs], mjl[:, cs]))
        th.append(lambda: nc.vector.tensor_copy(mu2[:, cs], m2[:, cs]))
        # assemble dist (fp16): base = diagv_i everywhere, then patch
        th.append(lambda: nc.scalar.activation(dist[:, cs], mju[:, cs], AF.Identity, bias=DVi, scale=0.0))
        th.append(lambda: nc.vector.copy_predicated(dist[:, cs], mu1[:, cs], d1p[:, cs]))
        th.append(lambda: nc.vector.copy_predicated(dist[:, cs], mu2[:, cs], d2p[:, cs]))
        return th

    # ---- phase 3: DMA-XBAR transposes + relayout into dT18 ----
    def ph3_thunks(half):
        th = []
        for tt in range(4 * half, 4 * half + 4):
            th.append(lambda tt=tt: nc.sync.dma_start_transpose(
                dT_sb[:, tt * 128:(tt + 1) * 128],
                dist[:, tt * 128:(tt + 1) * 128]))
        # relayout: dT18[q, (8g+b)*128 + i] = dT_sb[16b+q, 128g + i]
        dT18_v = dT18[0:16, :].rearrange("p (hh i) -> p hh i", i=128)
        dT_sb_v = dT_sb[:, :].rearrange("p (g i) -> p g i", i=128)
        gsl = slice(4 * half, 4 * half + 4)
        for b in range(8):
            hh0 = 32 * half + b
            th.append(lambda b=b, hh0=hh0: nc.sync.dma_start(
                dT18_v[:, hh0:hh0 + 25:8, :], dT_sb_v[16 * b:16 * (b + 1), gsl, :]))
        return th

    # first j-half fully before main loop
    for f in ph2_thunks(slice(0, 512)):
        f()
    for f in ph3_thunks(0):
        f()
    nc.sync.dma_start(dT18[16:17, :], t["dvi_rep"])
    nc.sync.dma_start(dT18[17:18, :], t["ones_row"])
    # second j-half + its transposes: interleaved under main-loop windows
    pending = deque(ph2_thunks(slice(512, 1024)))
    pending.extend(ph3_thunks(1))

    # ---- phase 4: main loop over 64 16-j windows, grouped by WB ----
    ps_logit = ctx.enter_context(tc.tile_pool(name="ps_logit", bufs=4, space="PSUM"))
    sig_pool = ctx.enter_context(tc.tile_pool(name="sig", bufs=3))
    out_pool = ctx.enter_context(tc.tile_pool(name="outsb", bufs=3))

    def fetch_group(g):
        nc.sync.dma_start(rhs18_bufs[g % 3][17:18, :], t["B_rows"][g:g + 1, :])
        nc.sync.dma_start(ahb_bufs[g % 3][:],
                          t["AH_rows"][g:g + 1, :].broadcast_to([128, WB * WFREE]))

    fetch_group(0)
    fetch_group(1)
    out_sb = None
    for hh in range(NW):
        g, wi = hh // WB, hh % WB
        rb = rhs18_bufs[g % 3]
        ahb = ahb_bufs[g % 3]
        if wi == 0:
            out_sb = out_pool.tile([R, WB * WFREE], F16, tag="out_sb",
                                   name="out_sb")
            if g + 2 < NG:
                fetch_group(g + 2)
        # drain a few prologue thunks per window (phase-2 second half)
        for _ in range(3):
            if pending:
                pending.popleft()()

        lg = ps_logit.tile([R, WFREE], F32, tag="lg", name="lg")
        for w in range(2):
            dst = slice(w * 512, (w + 1) * 512)
            wsrc = slice(wi * WFREE + w * 512, wi * WFREE + (w + 1) * 512)
            nc.tensor.matmul(lg[:, dst], dT18[0:18, hh * 128:(hh + 1) * 128],
                             rb[:, wsrc], start=True, stop=True)

        sig = sig_pool.tile([R, WFREE], F32, tag="sig", name="sig")
        nc.scalar.activation(sig[:], lg[:], AF.Sigmoid)

        seg = slice(wi * WFREE, (wi + 1) * WFREE)
        nc.vector.tensor_mul(out_sb[:, seg], sig[:], ahb[:, seg])
        if wi == WB - 1:
            base = (hh - (WB - 1)) * WFREE
            nc.sync.dma_start(t["out"][:, base:base + WB * WFREE], out_sb[:])

    while pending:
        pending.popleft()()
    if os.environ.get("KDBG"):
        nc.sync.dma_start(t["dbg_dist"], dist[:])
        nc.sync.dma_start(t["dbg_dT18"], dT18[:])


def build_nc():
    nc = bacc.Bacc("TRN2", target_bir_lowering=False, debug=False,
                   enable_asserts=False, num_devices=NCORES)
    t = _declare_tensors(nc)
    with tile.TileContext(nc) as tc:
        with ExitStack() as ctx:
            _build_program(ctx, tc, t)
    nc.compile()
    return nc


def kernel(**inputs):
    prep = _host_prep(**inputs)
    nc = build_nc()
    in_maps = [_core_inputs(prep, c) for c in range(NCORES)]
    res = bass_utils.run_bass_kernel_spmd(nc, in_maps, core_ids=list(range(NCORES)))
    out = np.concatenate([np.asarray(r["out"], np.float32)
                          for r in res.results], 0).reshape(N, N, H)
    out *= (1.0 / 256.0)
    # patch the diagonal (host-computed, uses GH and the diag logit)
    out[np.arange(N), np.arange(N)] = prep["out_diag"]
    return out


if __name__ == "__main__":
    import reference
    inputs = {k: np.asarray(v) for k, v in reference.setup_inputs().items()}
    out = kernel(**inputs)
    print("kernel out", out.shape, out.dtype)


# revision 63
# speedup vs baseline: 1.9181x; 1.0636x over previous
"""Trainium2 Bass kernel for nn_InteractionGate (gnn_message_passing).

Contract: kernel(**inputs) takes the FULL unsharded inputs (as in
reference.setup_inputs()) and returns the FULL [1024, 1024, 64] output.
Internally shards the pairwise row dimension i across 8 NeuronCores
(128 rows each), runs one SPMD Bass/Tile program on cores 0-7, gathers.

Math: with
  W1 = w_gate[0:64], W2 = w_gate[64:128], W3 = w_gate[128:144], W4 = w_gate[144:160]
  u3 = w_dist @ W3, u4 = w_dist @ W4
  B  = AH @ (W1+W2) + b_dist @ (W3+W4) + b_gate          [N,H]
the reference reduces (off-diagonal) to
  out[i,j,h] = AH[j,h] * sigmoid(B[j,h] + diagv[i]*u3[h] + dist[i,j]*u4[h])
where dist is the cal_dist "distance_other" matrix. The diagonal entries
use GH instead of AH and are patched on the host (O(N*H) work).

Device plan per core (rows i in its 128-block, partitions = i):
  1. PE computes the five pairwise numerator matrices (each is rank<=6:
     sum_k f_k(i) g_k(j)) as K=6 fp32 matmuls (cancellation-sensitive).
  2. DVE/ACT/Pool compute dist[i,j] [128,1024] elementwise in two j-halves
     (fp32 for the cancellation-sensitive intersection points, fp16 for the
     distance chain), the second half interleaved under the main loop.
  3. DMA-XBAR transposes dist (fp16) into dT_sb [128, 1024]
     (dT_sb[p, t*128+i] = dist[i, 128t+p]; no PE/PSUM), then relayout
     DMAs build dT18 [18, 64*128]: dT18[q, hh*128+i] = dist[i, 16hh+q],
     row 16 = diagv rep, row 17 = ones.
  4. Main loop over 64 16-j windows (1024 free cols, all-fp16 PE):
     PE:  logit = dT18[:, hh*128:+128].T @ rhs18  (K=18: u4-delta rows,
          u3 row, B row -- B DMA'd into row 17 per 4-window group)
     ACT: sig = sigmoid(logit)  (PSUM -> SBUF fp16; lg pool bufs=4)
     DVE: out16 = sig * ahb     (all-fp16 SBUF, 2x mode)
     ahb = AH rows broadcast across partitions by DMA (per group)
     DMA: out fp16 (512 KiB per group) -> HBM; host upcasts to fp32.
"""
import os
import sys
from collections import deque
from contextlib import ExitStack

import numpy as np

if "/opt/trn_rl_repo" not in sys.path:
    sys.path.insert(0, "/opt/trn_rl_repo")

import concourse.bass as bass
import concourse.bacc as bacc
import concourse.mybir as mybir
import concourse.tile as tile
from concourse import bass_utils

N, H, E = 1024, 64, 16
NCORES = 8
R = N // NCORES            # 128 rows per core
F32 = mybir.dt.float32
F16 = mybir.dt.float16
U8 = mybir.dt.uint8
BF16 = mybir.dt.bfloat16
AF = mybir.ActivationFunctionType
OP = mybir.AluOpType

NJW = 16                   # j's per window
WFREE = NJW * H            # 1024 free elements per window
NW = N // NJW              # 64 windows
WB = 4                     # windows per group (buffers + out DMA batch)
NG = NW // WB              # 16 groups


def _sigmoid(x):
    return 1.0 / (1.0 + np.exp(-x))


def jnp_bf16(x):
    import ml_dtypes
    return np.asarray(x, np.float32).astype(ml_dtypes.bfloat16)


def _host_prep(action_hidden_state, goal_hidden_state, goal, action,
               w_dist, b_dist, w_gate, b_gate):
    f32, f16 = np.float32, np.float16
    AH = np.ascontiguousarray(action_hidden_state, f32)
    GH = np.ascontiguousarray(goal_hidden_state, f32)
    goal = np.asarray(goal, f32)
    action = np.asarray(action, f32)
    w_dist = np.asarray(w_dist, f32)
    b_dist = np.asarray(b_dist, f32)
    w_gate = np.asarray(w_gate, f32)
    b_gate = np.asarray(b_gate, f32)

    ax, ay = action[:, 0].copy(), action[:, 1].copy()
    gx, gy = goal[:, 0].copy(), goal[:, 1].copy()
    gyx = gy - gx
    diagv = np.sqrt((ax - gx) ** 2 + (ay - gy) ** 2).astype(f32)

    W1, W2 = w_gate[0:H], w_gate[H:2 * H]
    W3, W4 = w_gate[2 * H:2 * H + E], w_gate[2 * H + E:2 * H + 2 * E]
    u3 = (w_dist @ W3).astype(f32)
    u4 = (w_dist @ W4).astype(f32)
    B = (AH @ (W1 + W2) + b_dist @ (W3 + W4) + b_gate).astype(f32)

    one = np.ones(N, f32)
    # rank factors: num[i,j] = sum_k f[k][i] * g[k][j]
    f_cav = np.stack([ax, -ax * gx, -ay, ay * gx])
    g_cav = np.stack([ay * gx, ay, ax * gx, ax])
    f_caz = np.stack([ax, -ax * gy, -ay, ay * gy])
    g_caz = np.stack([ay * gy, ay, ax * gy, ax])
    f_wcg1 = np.stack([gx, -ax * gx]); g_wcg1 = np.stack([ax * gyx, gyx])
    f_wcg2 = np.stack([gyx, -ax * gyx]); g_wcg2 = np.stack([ax * gx, gx])
    f_scg1 = np.stack([gx, -ay * gx]); g_scg1 = np.stack([ax * gyx, gyx])
    f_t2 = np.stack([gyx, -ax * gyx]); g_t2 = np.stack([ay * gx, gx])
    f_dnm = np.stack([one, -ay, -gx, ay * gx, np.zeros(N, f32), np.zeros(N, f32)])
    g_dnm = np.stack([ay * gx, gx, ay, one, np.zeros(N, f32), np.zeros(N, f32)])

    fg = dict(
        dnm=(f_dnm, g_dnm),
        num1=(np.concatenate([f_cav, -f_wcg1]), np.concatenate([g_cav, g_wcg1])),
        num1p=(np.concatenate([f_cav, f_wcg2]), np.concatenate([g_cav, g_wcg2])),
        num2=(np.concatenate([f_caz, -f_scg1]), np.concatenate([g_caz, g_scg1])),
        num2p=(np.concatenate([f_caz, f_t2]), np.concatenate([g_caz, g_t2])),
    )

    logit_diag = (B + (GH - AH) @ W2 + diagv[:, None] * (u3 + u4)).astype(f32)
    out_diag = (GH * _sigmoid(logit_diag)).astype(f32)

    # rhs18 static rows 0..16: u4-delta for a 16-j window + u3 row, tiled WB x
    rhs18s = np.zeros((17, WFREE), f32)
    for q in range(NJW):
        rhs18s[q, q * H:(q + 1) * H] = u4
    rhs18s[16] = np.tile(u3, NJW)
    rhs18s = np.tile(rhs18s, (1, WB)).astype(f16)           # [17, WB*WFREE]

    B_rows = B.reshape(NG, WB * WFREE).astype(f16)          # [16, 4096]
    # AH scaled by 256 so out*256 stays in fp16 normal range (the host
    # divides it back out); kills the fp16-subnormal precision cliff.
    AH_rows = (AH.reshape(NG, WB * WFREE) * 256.0).astype(f16)  # [16, 4096]

    return dict(AH=AH, GH=GH, ax=ax, ay=ay, gx=gx, gy=gy, diagv=diagv,
                u3=u3, u4=u4, B=B, fg=fg, out_diag=out_diag,
                rhs18s=rhs18s, B_rows=B_rows, AH_rows=AH_rows)


NUM_NAMES = ["dnm", "num1", "num1p", "num2", "num2p"]


def _core_inputs(prep, core):
    """Build the per-core in_map (numpy arrays for every ExternalInput)."""
    f32, f16 = np.float32, np.float16
    i0 = core * R
    sl = slice(i0, i0 + R)

    sc = np.zeros((R, 8), f32)
    sc[:, 0] = prep["ax"][sl]
    sc[:, 1] = prep["ay"][sl]
    sc[:, 2] = prep["gx"][sl]
    sc[:, 3] = prep["diagv"][sl]

    jj = np.arange(N)[None, :]
    ii = np.arange(i0, i0 + R)[:, None]
    mju = (jj > ii).astype(f32)
    mjl = (jj < ii).astype(f32)
    meye = (jj == ii).astype(f32)

    axj_b = np.broadcast_to(prep["ax"], (R, N)).copy()
    gxj_b = np.broadcast_to(prep["gx"], (R, N)).copy()

    dvi_rep = np.tile(prep["diagv"][sl], NW)[None, :].astype(f16)  # [1, 64*128]
    ones_row = np.ones((1, NW * 128), f16)

    m = dict(sc=sc, mju=mju, mjl=mjl, meye=meye, axj_b=axj_b, gxj_b=gxj_b,
             dvi_rep=dvi_rep, ones_row=ones_row, rhs18s=prep["rhs18s"],
             B_rows=prep["B_rows"], AH_rows=prep["AH_rows"])
    for nm in NUM_NAMES:
        f, g = prep["fg"][nm]
        m[f"lhsT_{nm}"] = np.ascontiguousarray(f[:, sl].astype(f32))  # [6, 128]
        m[f"rhs_{nm}"] = np.ascontiguousarray(g.astype(f32))          # [6, 1024]
    return m


def _declare_tensors(nc):
    t = {}
    def inp(name, shape, dt=F32):
        t[name] = nc.dram_tensor(name, shape, dt, kind="ExternalInput").ap()
    inp("sc", [R, 8])
    inp("mju", [R, N]); inp("mjl", [R, N]); inp("meye", [R, N])
    inp("axj_b", [R, N]); inp("gxj_b", [R, N])
    inp("dvi_rep", [1, NW * 128], F16)
    inp("ones_row", [1, NW * 128], F16)
    inp("rhs18s", [17, WB * WFREE], F16)
    inp("B_rows", [NG, WB * WFREE], F16)
    inp("AH_rows", [NG, WB * WFREE], F16)
    for nm in NUM_NAMES:
        inp(f"lhsT_{nm}", [6, 128])
        inp(f"rhs_{nm}", [6, N])
    t["out"] = nc.dram_tensor("out", [R, N * H], F16, kind="ExternalOutput").ap()
    if os.environ.get("KDBG"):
        t["dbg_dist"] = nc.dram_tensor("dbg_dist", [R, N], F16, kind="ExternalOutput").ap()
        t["dbg_dT18"] = nc.dram_tensor("dbg_dT18", [18, NW * 128], F16, kind="ExternalOutput").ap()
    return t


def _build_program(ctx, tc, t):
    nc = tc.nc

    consts = ctx.enter_context(tc.tile_pool(name="consts", bufs=1))

    def load_in(pool, name, shape, dt=F32, eng=None):
        tl = pool.tile(shape, dt, tag=name, name=name)
        (eng or nc.sync).dma_start(tl[:], t[name])
        return tl

    sc = load_in(consts, "sc", [R, 8])
    dT18 = consts.tile([18, NW * 128], F16, tag="dT18", name="dT18")
    dT_sb = consts.tile([128, N], F16, tag="dT_sb", name="dT_sb")
    rhs18_bufs = []
    ahb_bufs = []
    for bi in range(3):
        rb = consts.tile([18, WB * WFREE], F16, tag=f"rhs18_{bi}",
                         name=f"rhs18_{bi}")
        nc.sync.dma_start(rb[0:17, :], t["rhs18s"])
        rhs18_bufs.append(rb)
        ab = consts.tile([128, WB * WFREE], F16, tag=f"ahb_{bi}",
                         name=f"ahb_{bi}")
        ahb_bufs.append(ab)

    # prologue pool: everything phases 1-3 need
    pro = ctx.enter_context(tc.tile_pool(name="pro", bufs=1))
    lhsT_num = {nm: load_in(pro, f"lhsT_{nm}", [6, 128], eng=nc.scalar)
                for nm in NUM_NAMES}
    rhs_num = {nm: load_in(pro, f"rhs_{nm}", [6, N], eng=nc.scalar)
               for nm in NUM_NAMES}
    meye = load_in(pro, "meye", [R, N])
    mju = load_in(pro, "mju", [R, N])
    mjl = load_in(pro, "mjl", [R, N])
    axj_b = load_in(pro, "axj_b", [R, N])
    gxj_b = load_in(pro, "gxj_b", [R, N])

    AXi, AYi, GXi, DVi = (sc[:, k:k + 1] for k in range(4))

    # ---- phase 1: numerators via PE (rank<=6, fp32), eviction to SBUF ----
    num_sb = {}
    ps_num_cm = tc.tile_pool(name="ps_num", bufs=2, space="PSUM")
    ps_num = ps_num_cm.__enter__()
    for nm in NUM_NAMES:
        ps = ps_num.tile([R, N], F32, tag="ps_num", name="ps_num")
        for w in range(N // 512):
            nc.tensor.matmul(ps[:, w * 512:(w + 1) * 512],
                             lhsT_num[nm][:, :],
                             rhs_num[nm][:, w * 512:(w + 1) * 512],
                             start=True, stop=True)
        sb = pro.tile([R, N], F32, tag=f"num_{nm}", name=f"num_{nm}")
        nc.scalar.copy(sb[:], ps[:])
        num_sb[nm] = sb
    ps_num_cm.__exit__(None, None, None)

    # ---- phase 2: dist [128, cs] elementwise, per column-slice ----
    p1, p2, p1p, p2p = (num_sb[k] for k in ("num1", "num2", "num1p", "num2p"))
    rdn = num_sb["dnm"]

    def mk(tag, dt):
        return pro.tile([R, N], dt, tag=tag, name=tag)

    e1 = mk("e1", F32); q1 = mk("q1", F32)
    e1s = mk("e1s", F32); e2s = mk("e2s", F32)
    s12 = mk("s12", F32); d1p = mk("d1p", F16); d2p = mk("d2p", F16)
    t1 = mk("t1", F32); t2 = mk("t2", F32)
    m1 = mk("m1", F32)
    q2 = q1; m2 = m1  # branch-2 aliases (branch-1 uses precede)
    mu1 = mk("mu1", U8); mu2 = mk("mu2", U8)
    dist = mk("dist", F16)

    def ph2_thunks(cs):
        """Elementwise dist computation on column slice cs, as thunk list."""
        th = []
        # intersection points (fp32, cancellation-sensitive)
        th.append(lambda: nc.vector.tensor_add(rdn[:, cs], rdn[:, cs], meye[:, cs]))
        th.append(lambda: nc.vector.reciprocal(rdn[:, cs], rdn[:, cs]))
        th.append(lambda: nc.vector.tensor_mul(p1[:, cs], p1[:, cs], rdn[:, cs]))
        th.append(lambda: nc.vector.tensor_mul(p2[:, cs], p2[:, cs], rdn[:, cs]))
        th.append(lambda: nc.gpsimd.tensor_mul(p1p[:, cs], p1p[:, cs], rdn[:, cs]))
        th.append(lambda: nc.gpsimd.tensor_mul(p2p[:, cs], p2p[:, cs], rdn[:, cs]))
        # upper-triangle branch: d1p = |a_i - p|, cond (ax-p1)(p1-gx)>0
        th.append(lambda: nc.vector.tensor_scalar(e1[:, cs], p1[:, cs], AXi, None, OP.subtract))
        th.append(lambda: nc.vector.scalar_tensor_tensor(q1[:, cs], p1[:, cs], GXi, e1[:, cs], OP.subtract, OP.mult))
        th.append(lambda: nc.scalar.square(e1s[:, cs], e1[:, cs]))
        th.append(lambda: nc.vector.tensor_scalar(e1[:, cs], p2[:, cs], AYi, None, OP.subtract))
        th.append(lambda: nc.scalar.square(e2s[:, cs], e1[:, cs]))
        th.append(lambda: nc.vector.tensor_add(s12[:, cs], e1s[:, cs], e2s[:, cs]))
        th.append(lambda: nc.scalar.sqrt(d1p[:, cs], s12[:, cs]))
        th.append(lambda: nc.vector.tensor_scalar(m1[:, cs], q1[:, cs], 0.0, None, OP.is_lt))
        th.append(lambda: nc.gpsimd.tensor_mul(m1[:, cs], m1[:, cs], mju[:, cs]))
        th.append(lambda: nc.vector.tensor_copy(mu1[:, cs], m1[:, cs]))
        # lower-triangle branch: d2p = |a_i - p'|, cond (axj-p1')(p1'-gxj)>0
        th.append(lambda: nc.vector.tensor_scalar(e1[:, cs], p1p[:, cs], AXi, None, OP.subtract))
        th.append(lambda: nc.scalar.square(e1s[:, cs], e1[:, cs]))
        th.append(lambda: nc.vector.tensor_scalar(e1[:, cs], p2p[:, cs], AYi, None, OP.subtract))
        th.append(lambda: nc.scalar.square(e2s[:, cs], e1[:, cs]))
        th.append(lambda: nc.vector.tensor_add(s12[:, cs], e1s[:, cs], e2s[:, cs]))
        th.append(lambda: nc.scalar.sqrt(d2p[:, cs], s12[:, cs]))
        th.append(lambda: nc.vector.tensor_sub(t1[:, cs], p1p[:, cs], axj_b[:, cs]))
        th.append(lambda: nc.gpsimd.tensor_sub(t2[:, cs], p1p[:, cs], gxj_b[:, cs]))
        th.append(lambda: nc.vector.tensor_mul(q2[:, cs], t1[:, cs], t2[:, cs]))
        th.append(lambda: nc.vector.tensor_scalar(m2[:, cs], q2[:, cs], 0.0, None, OP.is_lt))
        th.append(lambda: nc.gpsimd.tensor_mul(m2[:, cs], m2[:, cs], mjl[:, cs]))
        th.append(lambda: nc.vector.tensor_copy(mu2[:, cs], m2[:, cs]))
        # assemble dist (fp16): base = diagv_i everywhere, then patch
        th.append(lambda: nc.scalar.activation(dist[:, cs], mju[:, cs], AF.Identity, bias=DVi, scale=0.0))
        th.append(lambda: nc.vector.copy_predicated(dist[:, cs], mu1[:, cs], d1p[:, cs]))
        th.append(lambda: nc.vector.copy_predicated(dist[:, cs], mu2[:, cs], d2p[:, cs]))
        return th

    # ---- phase 3: DMA-XBAR transposes + relayout into dT18 ----
    def ph3_thunks(half):
        th = []
        for tt in range(4 * half, 4 * half + 4):
            th.append(lambda tt=tt: nc.sync.dma_start_transpose(
                dT_sb[:, tt * 128:(tt + 1) * 128],
                dist[:, tt * 128:(tt + 1) * 128]))
        # relayout: dT18[q, (8g+b)*128 + i] = dT_sb[16b+q, 128g + i]
        dT18_v = dT18[0:16, :].rearrange("p (hh i) -> p hh i", i=128)
        dT_sb_v = dT_sb[:, :].rearrange("p (g i) -> p g i", i=128)
        gsl = slice(4 * half, 4 * half + 4)
        for b in range(8):
            hh0 = 32 * half + b
            th.append(lambda b=b, hh0=hh0: nc.sync.dma_start(
                dT18_v[:, hh0:hh0 + 25:8, :], dT_sb_v[16 * b:16 * (b + 1), gsl, :]))
        return th

    def fetch_group(g):
        nc.sync.dma_start(rhs18_bufs[g % 3][17:18, :], t["B_rows"][g:g + 1, :])
        nc.sync.dma_start(ahb_bufs[g % 3][:],
                          t["AH_rows"][g:g + 1, :].broadcast_to([128, WB * WFREE]))

    # dependency-free DMAs first: main-loop group prefetches + dT18 rows run
    # on the DMA engines while the compute engines chew on phases 1-2
    nc.sync.dma_start(dT18[16:17, :], t["dvi_rep"])
    nc.sync.dma_start(dT18[17:18, :], t["ones_row"])
    fetch_group(0)
    fetch_group(1)

    # first j-half fully before main loop
    for f in ph2_thunks(slice(0, 512)):
        f()
    for f in ph3_thunks(0):
        f()
    # second j-half + its transposes: interleaved under main-loop windows
    pending = deque(ph2_thunks(slice(512, 1024)))
    pending.extend(ph3_thunks(1))

    # ---- phase 4: main loop over 64 16-j windows, grouped by WB ----
    ps_logit = ctx.enter_context(tc.tile_pool(name="ps_logit", bufs=4, space="PSUM"))
    sig_pool = ctx.enter_context(tc.tile_pool(name="sig", bufs=3))
    out_pool = ctx.enter_context(tc.tile_pool(name="outsb", bufs=3))
    out_sb = None
    for hh in range(NW):
        g, wi = hh // WB, hh % WB
        rb = rhs18_bufs[g % 3]
        ahb = ahb_bufs[g % 3]
        if wi == 0:
            out_sb = out_pool.tile([R, WB * WFREE], F16, tag="out_sb",
                                   name="out_sb")
            if g + 2 < NG:
                fetch_group(g + 2)
        # drain a few prologue thunks per window (phase-2 second half)
        for _ in range(4):
            if pending:
                pending.popleft()()

        lg = ps_logit.tile([R, WFREE], F32, tag="lg", name="lg")
        for w in range(2):
            dst = slice(w * 512, (w + 1) * 512)
            wsrc = slice(wi * WFREE + w * 512, wi * WFREE + (w + 1) * 512)
            nc.tensor.matmul(lg[:, dst], dT18[0:18, hh * 128:(hh + 1) * 128],
                             rb[:, wsrc], start=True, stop=True)

        sig = sig_pool.tile([R, WFREE], F32, tag="sig", name="sig")
        nc.scalar.activation(sig[:], lg[:], AF.Sigmoid)

        seg = slice(wi * WFREE, (wi + 1) * WFREE)
        nc.vector.tensor_mul(out_sb[:, seg], sig[:], ahb[:, seg])
        if wi == WB - 1:
            base = (hh - (WB - 1)) * WFREE
            nc.sync.dma_start(t["out"][:, base:base + WB * WFREE], out_sb[:])

    while pending:
        pending.popleft()()
    if os.environ.get("KDBG"):
        nc.sync.dma_start(t["dbg_dist"], dist[:])
        nc.sync.dma_start(t["dbg_dT18"], dT18[:])


def build_nc():
    nc = bacc.Bacc("TRN2", target_bir_lowering=False, debug=False,
                   enable_asserts=False, num_devices=NCORES)
    t = _declare_tensors(nc)
    with tile.TileContext(nc) as tc:
        with ExitStack() as ctx:
            _build_program(ctx, tc, t)
    nc.compile()
    return nc


def kernel(**inputs):
    prep = _host_prep(**inputs)
    nc = build_nc()
    in_maps = [_core_inputs(prep, c) for c in range(NCORES)]
    res = bass_utils.run_bass_kernel_spmd(nc, in_maps, core_ids=list(range(NCORES)))
    out = np.concatenate([np.asarray(r["out"], np.float32)
                          for r in res.results], 0).reshape(N, N, H)
    out *= (1.0 / 256.0)
    # patch the diagonal (host-computed, uses GH and the diag logit)
    out[np.arange(N), np.arange(N)] = prep["out_diag"]
    return out


if __name__ == "__main__":
    import reference
    inputs = {k: np.asarray(v) for k, v in reference.setup_inputs().items()}
    out = kernel(**inputs)
    print("kernel out", out.shape, out.dtype)
